# revision 1
# baseline (speedup 1.0000x reference)
"""Trainium2 Bass kernel for a dense transformer block (pre-LN, causal attention, GELU FFN).

Sharding: 8 cores = 2 batches x 4 query-groups of 512 tokens. Every core
computes full K/V for its batch (communication-free); queries/proj/FFN are
token-parallel. All activations are kept feature-major ([d, tokens]) so no
on-device transposes are needed; LayerNorm is folded into host-prepared
weights plus on-device per-token column stats applied at PSUM evacuation.
"""

import sys

sys.path.insert(0, "/opt/trn_rl_repo")

import numpy as np
import ml_dtypes

import concourse.bass as bass
import concourse.tile as tile
from concourse import bacc, mybir
from concourse.bass import ts
from concourse.bass_utils import run_bass_kernel_spmd

F32 = mybir.dt.float32
F32R = mybir.dt.float32r
BF16 = mybir.dt.bfloat16
AF = mybir.ActivationFunctionType
ALU = mybir.AluOpType

EPS = 1e-5


class CFG:
    def __init__(self, D=1024, TB=2048, TQ=512, NH=16, HD=64, HFF=4096):
        self.D, self.TB, self.TQ, self.NH, self.HD, self.HFF = D, TB, TQ, NH, HD, HFF
        self.DT = D // 128          # d_model tiles
        self.FT = HFF // 128        # ffn tiles
        self.NTT = TB // 128        # key token tiles
        self.NBLK = TB // 512       # 512-token kv blocks
        self.VN = min(512, D)       # V matmul free width
        self.NVB = D // self.VN     # V col blocks
        self.HPV = self.VN // HD    # heads per V col block
        assert NH == 2 * self.DT and HD == 64




def build_nc(c: CFG):
    nc = bacc.Bacc()
    D, TB, TQ, DT, FT, NTT, NBLK = c.D, c.TB, c.TQ, c.DT, c.FT, c.NTT, c.NBLK
    XW = 256                      # x-stream block width
    NXB = TB // XW

    xT = nc.dram_tensor("xT", [D, TB], F32R, kind="ExternalInput")
    xqT = nc.dram_tensor("xqT", [D, TQ], F32R, kind="ExternalInput")
    maskT = nc.dram_tensor("maskT", [TB, TQ], BF16, kind="ExternalInput")
    wq = nc.dram_tensor("wq", [128, DT, D], BF16, kind="ExternalInput")
    wk = nc.dram_tensor("wk", [128, DT, D], BF16, kind="ExternalInput")
    wv = nc.dram_tensor("wv", [c.NVB, 128, DT, c.VN], BF16, kind="ExternalInput")
    pw = nc.dram_tensor("pw", [128, DT, D], F32R, kind="ExternalInput")
    w1 = nc.dram_tensor("w1", [FT // 2, 128, DT, 256], F32R,
                        kind="ExternalInput")
    w2 = nc.dram_tensor("w2", [DT, 128, FT, 128], F32R, kind="ExternalInput")
    bq = nc.dram_tensor("bq", [128, DT], F32, kind="ExternalInput")
    bk = nc.dram_tensor("bk", [128, DT], F32, kind="ExternalInput")
    bv = nc.dram_tensor("bv", [1, D], F32, kind="ExternalInput")
    pb = nc.dram_tensor("pb", [128, DT], F32, kind="ExternalInput")
    b1 = nc.dram_tensor("b1", [128, FT], F32, kind="ExternalInput")
    b2 = nc.dram_tensor("b2", [128, DT], F32, kind="ExternalInput")
    outT = nc.dram_tensor("outT", [D, TQ], F32, kind="ExternalOutput")

    def dram3(t):  # [ (a p), m ] -> [p, a, m]
        return t.ap().rearrange("(a p) m -> p a m", p=128)

    with tile.TileContext(nc) as tc:
        with tc.tile_pool(name="persist", bufs=1) as P:
            aoT = P.tile([128, DT, TQ], F32R)
            x2T = P.tile([128, DT, TQ], F32R)
            bvb = P.tile([128, D], F32)
            r_row = P.tile([1, TB], F32)
            rt = P.tile([128, NTT], F32)
            ones = P.tile([128, 1], F32R)
            eps_t = P.tile([1, 1], F32)
            id11 = P.tile([1, 1], F32)
            bq_t = P.tile([128, DT], F32)
            bk_t = P.tile([128, DT], F32)
            pb_t = P.tile([128, DT], F32)
            b2_t = P.tile([128, DT], F32)
            b1_t = P.tile([128, FT], F32)

            ones_f = P.tile([128, 1], F32)
            nc.vector.memset(ones_f[:], 1.0)
            nc.vector.tensor_copy(ones[:], ones_f[:])
            nc.vector.memset(eps_t[:], EPS)
            nc.vector.memset(id11[:], 1.0)
            nc.sync.dma_start(bq_t[:], bq.ap())
            nc.sync.dma_start(bk_t[:], bk.ap())
            nc.sync.dma_start(pb_t[:], pb.ap())
            nc.sync.dma_start(b2_t[:], b2.ap())
            nc.sync.dma_start(b1_t[:], b1.ap())

            # stats over the feature (partition) axis + centering, feature-major
            def stats_center(xsrc, W, r_slice, mu_bc, r_bc, dst, sqp, stp):
                sum_ps = stp.tile([1, W], F32, tag="sum_ps")
                sq_ps = stp.tile([1, W], F32, tag="sq_ps")
                for k in range(DT):
                    nc.tensor.matmul(sum_ps[:], ones[:], xsrc[:, k, :],
                                     start=(k == 0), stop=(k == DT - 1))
                for k in range(DT):
                    sq = sqp.tile([128, W], F32R, tag="sq")
                    nc.scalar.square(sq[:], xsrc[:, k, :])
                    nc.tensor.matmul(sq_ps[:], ones[:], sq[:],
                                     start=(k == 0), stop=(k == DT - 1))
                mu_r = sqp.tile([1, W], F32, tag="mu_r", bufs=1)
                nc.vector.tensor_scalar(mu_r[:], sum_ps[:], 1.0 / D, None, ALU.mult)
                musq = sqp.tile([1, W], F32, tag="musq", bufs=1)
                nc.vector.tensor_tensor(musq[:], mu_r[:], mu_r[:], ALU.mult)
                var = sqp.tile([1, W], F32, tag="var", bufs=1)
                nc.vector.scalar_tensor_tensor(var[:], sq_ps[:], 1.0 / D, musq[:],
                                               ALU.mult, ALU.subtract)
                std = sqp.tile([1, W], F32, tag="std", bufs=1)
                nc.scalar.activation(std[:], var[:], AF.Sqrt, bias=eps_t[:])
                nc.vector.reciprocal(r_slice, std[:])
                nc.gpsimd.partition_broadcast(mu_bc[:], mu_r[:])
                if r_bc is not None:
                    nc.gpsimd.partition_broadcast(r_bc[:], r_slice)
                for k in range(DT):
                    nc.vector.tensor_tensor(dst[:, k, :], xsrc[:, k, :], mu_bc[:],
                                            ALU.subtract)

            with tc.tile_pool(name="kvres", bufs=1) as KV:
                kT = KV.tile([128, DT, TB], BF16)
                V = KV.tile([128, NTT, c.NH * 65], BF16)
                qT = KV.tile([128, DT, TQ], BF16)
                vone = V[:].rearrange("p t (h c) -> p t h c", c=65)

                # ----- Phase X+K fused: stream/center x, K per ready 512-block -----
                with tc.tile_pool(name="phx", bufs=1) as XP:
                    xt = XP.tile([128, DT, TB], BF16)
                    with tc.tile_pool(name="phxs", bufs=2) as PX, \
                         tc.tile_pool(name="phxs_ps", bufs=2, space="PSUM") as PXP:
                        wk_t = PX.tile([128, DT, D], BF16, bufs=1)
                        for xi in range(NXB):
                            off = xi * XW
                            xb = PX.tile([128, DT, XW], F32R, tag="xb", bufs=2)
                            nc.sync.dma_start(xb[:], dram3(xT)[:, :, off:off + XW])
                            if xi == 0:
                                for kk in range(4):
                                    nc.sync.dma_start(
                                        wk_t[:, 2 * kk:2 * kk + 2, :],
                                        wk.ap()[:, 2 * kk:2 * kk + 2, :])
                            mu_bc = PX.tile([128, XW], F32, tag="mu_bc", bufs=2)
                            stats_center(xb, XW, r_row[0:1, off:off + XW],
                                         mu_bc, None, xt[:, :, off:off + XW],
                                         PX, PXP)
                            for tt in range(XW // 128):
                                g = off // 128 + tt
                                rt_ps = PXP.tile([128, 1], F32, tag="rt_ps")
                                nc.tensor.transpose(
                                    rt_ps[:],
                                    r_row[0:1, g * 128:(g + 1) * 128], id11[:])
                                nc.vector.tensor_copy(rt[:, g:g + 1], rt_ps[:])
                            if xi % 2 == 1:
                                b4 = xi // 2
                                off4 = b4 * 512
                                rb4 = PX.tile([128, 512], F32, tag="rb4", bufs=1)
                                nc.gpsimd.partition_broadcast(
                                    rb4[:], r_row[0:1, off4:off4 + 512])
                                for m in range(DT):
                                    ps = PXP.tile([128, 512], F32, tag="kps")
                                    for k in range(DT):
                                        nc.tensor.matmul(
                                            ps[:], wk_t[:, k, ts(m, 128)],
                                            xt[:, k, off4:off4 + 512],
                                            start=(k == 0), stop=(k == DT - 1))
                                    ev = PX.tile([128, 512], F32, tag="kev",
                                                 bufs=2)
                                    nc.vector.tensor_tensor(ev[:], ps[:], rb4[:],
                                                            ALU.mult)
                                    nc.vector.tensor_scalar(
                                        kT[:, m, off4:off4 + 512], ev[:],
                                        bk_t[:, m:m + 1], None, ALU.add)

                    nc.vector.memset(vone[:, :, :, 64:65], 1.0)
                    bv_r = XP.tile([1, D], F32)
                    nc.sync.dma_start(bv_r[:], bv.ap())
                    nc.gpsimd.partition_broadcast(bvb[:], bv_r[:])

                    # ---------------- Phase V ----------------
                    wvh0 = XP.tile([128, DT, c.VN], BF16)
                    for kk in range(4):
                        nc.sync.dma_start(wvh0[:, 2 * kk:2 * kk + 2, :],
                                          wv.ap()[0][:, 2 * kk:2 * kk + 2, :])
                    with tc.tile_pool(name="phv", bufs=2) as PV, \
                         tc.tile_pool(name="phv_ps", bufs=2, space="PSUM") as PVP:
                        for n in range(c.NVB):
                            if n == 0:
                                wvh = wvh0
                            else:
                                wvh = PV.tile([128, DT, c.VN], BF16,
                                              tag="wvh", bufs=1)
                                for kk in range(4):
                                    nc.sync.dma_start(
                                        wvh[:, 2 * kk:2 * kk + 2, :],
                                        wv.ap()[n][:, 2 * kk:2 * kk + 2, :])
                            for g in range(NTT):
                                ps = PVP.tile([128, c.VN], F32, tag="vps")
                                for k in range(DT):
                                    nc.tensor.matmul(ps[:], xt[:, k, ts(g, 128)],
                                                     wvh[:, k, :],
                                                     start=(k == 0),
                                                     stop=(k == DT - 1))
                                ev = PV.tile([128, c.VN], F32, tag="vev", bufs=3)
                                nc.vector.scalar_tensor_tensor(
                                    ev[:], ps[:], rt[:, g:g + 1],
                                    bvb[:, ts(n, c.VN)], ALU.mult, ALU.add)
                                dst = vone[:, g, n * c.HPV:(n + 1) * c.HPV, 0:64]
                                nc.vector.tensor_copy(
                                    dst, ev[:].rearrange("p (h c) -> p h c", c=64))

                # ---------------- Phase Q ----------------
                with tc.tile_pool(name="phq", bufs=2) as PQ, \
                     tc.tile_pool(name="phq_ps", bufs=2, space="PSUM") as PQP:
                    xq = PQ.tile([128, DT, TQ], F32R, bufs=1)
                    nc.sync.dma_start(xq[:], dram3(xqT))
                    xtq = PQ.tile([128, DT, TQ], BF16, bufs=1)
                    wq_t = PQ.tile([128, DT, D], BF16, bufs=1)
                    for kk in range(4):
                        nc.sync.dma_start(wq_t[:, 2 * kk:2 * kk + 2, :],
                                          wq.ap()[:, 2 * kk:2 * kk + 2, :])
                    rq_r = PQ.tile([1, TQ], F32, bufs=1)
                    muq_bc = PQ.tile([128, TQ], F32, bufs=1)
                    rq_bc = PQ.tile([128, TQ], F32, bufs=1)
                    stats_center(xq, TQ, rq_r[:], muq_bc, rq_bc, xtq, PQ, PQP)
                    for m in range(DT):
                        ps = PQP.tile([128, TQ], F32, tag="qps")
                        for k in range(DT):
                            nc.tensor.matmul(ps[:], wq_t[:, k, ts(m, 128)],
                                             xtq[:, k, :],
                                             start=(k == 0), stop=(k == DT - 1))
                        ev = PQ.tile([128, TQ], F32, tag="qev", bufs=2)
                        nc.vector.tensor_tensor(ev[:], ps[:], rq_bc[:], ALU.mult)
                        nc.vector.tensor_scalar(qT[:, m, :], ev[:],
                                                bq_t[:, m:m + 1], None, ALU.add)

                # ------------- Phase attention + proj -------------
                with tc.tile_pool(name="pha", bufs=2) as PA:
                    mk = PA.tile([128, NTT, TQ], BF16, bufs=1)
                    nc.sync.dma_start(mk[:], maskT.ap().rearrange(
                        "(a p) q -> p a q", p=128))
                    pw_t = PA.tile([128, DT, D], F32R, bufs=1)
                    for kk in range(4):
                        nc.sync.dma_start(pw_t[:, 2 * kk:2 * kk + 2, :],
                                          pw.ap()[:, 2 * kk:2 * kk + 2, :])
                    xq2 = PA.tile([128, DT, TQ], F32R, bufs=1)
                    nc.sync.dma_start(xq2[:], dram3(xqT))
                    with tc.tile_pool(name="pha_ps", bufs=2, space="PSUM") as PAP:
                        for hp in range(c.NH // 2):
                            av0 = PAP.tile([65, TQ], F32, tag="av0", bufs=2)
                            av1 = PAP.tile([65, TQ], F32, tag="av1", bufs=2)
                            h0, h1 = 2 * hp, 2 * hp + 1
                            pq = []

                            def do_av(tk, p01):
                                nc.tensor.matmul(av0[:],
                                                 V[:, tk, h0 * 65:(h0 + 1) * 65],
                                                 p01[:, 0, :], start=(tk == 0),
                                                 stop=(tk == NTT - 1))
                                nc.tensor.matmul(av1[:],
                                                 V[:, tk, h1 * 65:(h1 + 1) * 65],
                                                 p01[:, 1, :], start=(tk == 0),
                                                 stop=(tk == NTT - 1))

                            for tk in range(NTT):
                                s01 = PAP.tile([128, 2, TQ], F32, tag="s01",
                                               bufs=2)
                                nc.tensor.matmul(s01[:, 0, :],
                                                 kT[0:64, hp, ts(tk, 128)],
                                                 qT[0:64, hp, :],
                                                 start=True, stop=True)
                                nc.tensor.matmul(s01[:, 1, :],
                                                 kT[64:128, hp, ts(tk, 128)],
                                                 qT[64:128, hp, :],
                                                 start=True, stop=True)
                                e01 = PA.tile([128, 2, TQ], BF16, tag="e01",
                                              bufs=3)
                                nc.scalar.activation(e01[:], s01[:], AF.Exp)
                                p01 = PA.tile([128, 2, TQ], BF16, tag="p01",
                                              bufs=4)
                                mkb = mk[:, tk, :].unsqueeze(1).to_broadcast(
                                    (128, 2, TQ))
                                nc.vector.tensor_tensor(p01[:], e01[:], mkb,
                                                        ALU.mult)
                                pq.append((tk, p01))
                                if len(pq) > 1:
                                    do_av(*pq.pop(0))
                            do_av(*pq.pop(0))
                            for av, half in ((av0, 0), (av1, 1)):
                                rec = PA.tile([1, TQ], F32, tag="rec", bufs=2)
                                nc.vector.reciprocal(rec[:], av[64:65, :])
                                rbc = PA.tile([64, TQ], F32, tag="rbc", bufs=2)
                                nc.gpsimd.partition_broadcast(rbc[:], rec[:],
                                                              channels=64)
                                nc.vector.tensor_tensor(
                                    aoT[64 * half:64 * (half + 1), hp, :],
                                    av[0:64, :], rbc[:], ALU.mult)

                    with tc.tile_pool(name="php_ps", bufs=2, space="PSUM") as PPP:
                        for m in range(DT):
                            ps = PPP.tile([128, TQ], F32, tag="pps")
                            for k in range(DT):
                                nc.tensor.matmul(ps[:], pw_t[:, k, ts(m, 128)],
                                                 aoT[:, k, :],
                                                 start=(k == 0), stop=(k == DT - 1))
                            nc.vector.scalar_tensor_tensor(
                                x2T[:, m, :], ps[:], pb_t[:, m:m + 1],
                                xq2[:, m, :], ALU.add, ALU.add)

            # ---------------- Phase FFN ----------------
            with tc.tile_pool(name="phf", bufs=2) as PF, \
                 tc.tile_pool(name="phf_ps", bufs=2, space="PSUM") as PFP:
                h = PF.tile([128, FT, TQ], F32R, bufs=1)
                r2_r = PF.tile([1, TQ], F32, bufs=1)
                mu2_bc = PF.tile([128, TQ], F32, bufs=1)
                r2_bc = PF.tile([128, TQ], F32, bufs=1)
                stats_center(x2T, TQ, r2_r[:], mu2_bc, r2_bc, x2T, PF, PFP)
                for mg in range(FT // 2):
                    w1c = PF.tile([128, DT, 256], F32R, tag="w1c", bufs=4)
                    nc.sync.dma_start(w1c[:], w1.ap()[mg])
                    for mi in range(2):
                        m = 2 * mg + mi
                        ps = PFP.tile([128, TQ], F32, tag="hps")
                        for k in range(DT):
                            nc.tensor.matmul(ps[:], w1c[:, k, ts(mi, 128)],
                                             x2T[:, k, :],
                                             start=(k == 0), stop=(k == DT - 1))
                        th = PF.tile([128, TQ], F32, tag="th", bufs=2)
                        nc.vector.tensor_tensor(th[:], ps[:], r2_bc[:], ALU.mult)
                        nc.scalar.activation(h[:, m, :], th[:], AF.Gelu,
                                             bias=b1_t[:, m:m + 1])
                for m in range(DT):
                    w2c = PF.tile([128, FT, 128], F32R, tag="w2c", bufs=2)
                    for kk in range(4):
                        kw = FT // 4
                        nc.sync.dma_start(
                            w2c[:, kk * kw:(kk + 1) * kw, :],
                            w2.ap()[m][:, kk * kw:(kk + 1) * kw, :])
                    ps = PFP.tile([128, TQ], F32, tag="ops")
                    for k in range(FT):
                        nc.tensor.matmul(ps[:], w2c[:, k, :], h[:, k, :],
                                         start=(k == 0), stop=(k == FT - 1))
                    t2 = PF.tile([128, TQ], F32, tag="t2", bufs=2)
                    nc.vector.scalar_tensor_tensor(t2[:], ps[:], b2_t[:, m:m + 1],
                                                   x2T[:, m, :], ALU.add, ALU.add)
                    ob = PF.tile([128, TQ], F32, tag="ob", bufs=2)
                    nc.vector.tensor_tensor(ob[:], t2[:], mu2_bc[:], ALU.add)
                    nc.sync.dma_start(
                        outT.ap().rearrange("(a p) t -> p a t", p=128)[:, m, :],
                        ob[:])
    nc.compile()
    return nc


_CACHE = {}


def _get_nc(c: CFG):
    key = (c.D, c.TB, c.TQ, c.NH, c.HFF)
    if key not in _CACHE:
        _CACHE[key] = build_nc(c)
    return _CACHE[key]


def make_in_maps(c: CFG, x, mask, ln1_g, ln1_b, qkv_w, qkv_b, proj_w, proj_b,
                 ln2_g, ln2_b, w1, b1, w2, b2):
    D, TB, TQ, DT, FT = c.D, c.TB, c.TQ, c.DT, c.FT
    B = x.shape[0]
    ncg = TB // TQ  # query groups per batch

    f = np.float32
    bf = ml_dtypes.bfloat16
    g1 = ln1_g.astype(f)
    sc = 1.0 / np.sqrt(c.HD)
    DT, FT, VN, NVB = c.DT, c.FT, c.VN, c.NVB

    def tile_kp(w):  # [D, M] -> [128, DT, M] (partition-contiguous slabs)
        return np.ascontiguousarray(w.reshape(DT, 128, -1).transpose(1, 0, 2))

    wq_f = tile_kp((qkv_w[:, :D] * g1[:, None] * sc).astype(bf))
    wk_f = tile_kp((qkv_w[:, D:2 * D] * g1[:, None]).astype(bf))
    wv_b = (qkv_w[:, 2 * D:] * g1[:, None]).astype(bf)
    wv_f = np.ascontiguousarray(
        wv_b.reshape(DT, 128, NVB, VN).transpose(2, 1, 0, 3))
    bq_f = ((qkv_b[:D] + ln1_b @ qkv_w[:, :D]) * sc).astype(f)
    bk_f = (qkv_b[D:2 * D] + ln1_b @ qkv_w[:, D:2 * D]).astype(f)
    bv_f = (qkv_b[2 * D:] + ln1_b @ qkv_w[:, 2 * D:]).astype(f)
    w1g = (w1 * ln2_g.astype(f)[:, None]).astype(f)
    w1_f = np.ascontiguousarray(
        w1g.reshape(DT, 128, FT // 2, 256).transpose(2, 1, 0, 3))
    b1_f = (b1 + ln2_b @ w1).astype(f)
    pw_f = tile_kp(np.asarray(proj_w, f))
    w2_f = np.ascontiguousarray(
        np.asarray(w2, f).reshape(FT, 128, DT, 128).transpose(2, 1, 0, 3))

    def btile(v, nt):
        return np.ascontiguousarray(v.reshape(nt, 128).T, f)

    z01 = mask[0, 0].astype(np.float32)  # [T,T] 1=keep 0=drop

    shared = {
        "wq": wq_f, "wk": wk_f, "wv": wv_f, "pw": pw_f,
        "w1": w1_f, "w2": w2_f,
        "bq": btile(bq_f, DT), "bk": btile(bk_f, DT),
        "bv": np.ascontiguousarray(bv_f.reshape(1, D)),
        "pb": btile(proj_b.astype(f), DT),
        "b1": btile(b1_f, FT), "b2": btile(b2.astype(f), DT),
    }
    in_maps = []
    for core in range(B * ncg):
        b, j = core // ncg, core % ncg
        qs = j * TQ
        xTb = np.ascontiguousarray(x[b].T, f)                     # [D, TB]
        m = dict(shared)
        m["xT"] = xTb
        m["xqT"] = np.ascontiguousarray(x[b, qs:qs + TQ, :].T, f)  # [D, TQ]
        m["maskT"] = np.ascontiguousarray(z01[qs:qs + TQ, :].T.astype(bf))
        in_maps.append(m)
    return in_maps


def assemble_out(c: CFG, results, B):
    ncg = c.TB // c.TQ
    out = np.empty((B, c.TB, c.D), np.float32)
    for core, res in enumerate(results):
        b, j = core // ncg, core % ncg
        out[b, j * c.TQ:(j + 1) * c.TQ, :] = res["outT"].T
    return out


def kernel(x, mask, ln1_g, ln1_b, qkv_w, qkv_b, proj_w, proj_b,
           ln2_g, ln2_b, w1, b1, w2, b2):
    x = np.asarray(x, np.float32)
    c = CFG(D=x.shape[2], TB=x.shape[1], TQ=x.shape[1] // 4,
            NH=16, HD=64, HFF=4 * x.shape[2])
    nc = _get_nc(c)
    in_maps = make_in_maps(c, x, np.asarray(mask), *[np.asarray(a, np.float32)
                           for a in (ln1_g, ln1_b, qkv_w, qkv_b, proj_w, proj_b,
                                     ln2_g, ln2_b, w1, b1, w2, b2)])
    res = run_bass_kernel_spmd(nc, in_maps, core_ids=list(range(len(in_maps))))
    return assemble_out(c, res.results, x.shape[0])


if __name__ == "__main__":
    c = CFG()
    nc = build_nc(c)
    print("built ok")



# revision 55
# speedup vs baseline: 1.2616x; 1.2616x over previous
"""Trainium2 Bass kernel for a dense transformer block (pre-LN, causal attention, GELU FFN).

Sharding: 8 cores = 2 batches x 4 query-groups of 512 tokens, communication
free. Per-core the batch's token tiles are PERMUTED on host so that this
core's 4 query tiles sit at static positions {3,7,11,15} (within each group
of 4 tiles, the core's tile is moved to the group end, others stay in
ascending order). Keys stay causal-compatible: a query at position 4a+3 only
needs key positions 0..4a+3, so scores/AV run with causally truncated width
and only one diagonal 128x128 sub-tile per key tile needs mask application.
Q reuses the X-phase centered activations and row stats (no second LN pass).
All activations are feature-major [d, tokens]; LayerNorm is folded into
host-prepared weights plus per-token column stats applied at PSUM evacuation.
"""

import sys

sys.path.insert(0, "/opt/trn_rl_repo")

import numpy as np
import ml_dtypes

import concourse.bass as bass
import concourse.tile as tile
from concourse import bacc, mybir
from concourse.bass import ts
from concourse.bass_utils import run_bass_kernel_spmd

F32 = mybir.dt.float32
F32R = mybir.dt.float32r
BF16 = mybir.dt.bfloat16
AF = mybir.ActivationFunctionType
ALU = mybir.AluOpType

EPS = 1e-5


class CFG:
    def __init__(self, D=1024, TB=2048, TQ=512, NH=16, HD=64, HFF=4096):
        self.D, self.TB, self.TQ, self.NH, self.HD, self.HFF = D, TB, TQ, NH, HD, HFF
        self.DT = D // 128          # d_model tiles
        self.FT = HFF // 128        # ffn tiles
        self.NTT = TB // 128        # key token tiles
        self.NBLK = TB // 512       # 512-token kv blocks
        self.VN = min(512, D)       # V matmul free width
        self.NVB = D // self.VN     # V col blocks
        self.HPV = self.VN // HD    # heads per V col block
        assert NH == 2 * self.DT and HD == 64


def build_nc(c: CFG):
    nc = bacc.Bacc()
    D, TB, TQ, DT, FT, NTT, NBLK = c.D, c.TB, c.TQ, c.DT, c.FT, c.NTT, c.NBLK
    XW = 256                      # x-stream block width
    NXB = TB // XW
    NQT = TQ // 128               # query tiles per core

    xT = nc.dram_tensor("xT", [D, TB], F32R, kind="ExternalInput")
    xqT = nc.dram_tensor("xqT", [D, TQ], F32R, kind="ExternalInput")
    maskM = nc.dram_tensor("maskM", [128, NTT, 128], BF16, kind="ExternalInput")
    wq = nc.dram_tensor("wq", [128, DT, D], BF16, kind="ExternalInput")
    wk = nc.dram_tensor("wk", [128, DT, D], BF16, kind="ExternalInput")
    wv = nc.dram_tensor("wv", [c.NVB, 128, DT, c.VN], BF16, kind="ExternalInput")
    pw = nc.dram_tensor("pw", [128, DT, D], BF16, kind="ExternalInput")
    w1 = nc.dram_tensor("w1", [FT // 2, 128, DT, 256], BF16,
                        kind="ExternalInput")
    w2 = nc.dram_tensor("w2", [DT, 128, FT // 2, 256], BF16,
                        kind="ExternalInput")
    bq = nc.dram_tensor("bq", [128, DT], F32, kind="ExternalInput")
    bk = nc.dram_tensor("bk", [128, DT], F32, kind="ExternalInput")
    bv = nc.dram_tensor("bv", [1, D], BF16, kind="ExternalInput")
    pb = nc.dram_tensor("pb", [128, DT], F32, kind="ExternalInput")
    b1 = nc.dram_tensor("b1", [128, FT], F32, kind="ExternalInput")
    b2 = nc.dram_tensor("b2", [128, DT], F32, kind="ExternalInput")
    outT = nc.dram_tensor("outT", [D, TQ], F32, kind="ExternalOutput")

    def dram3(t):  # [ (a p), m ] -> [p, a, m]
        return t.ap().rearrange("(a p) m -> p a m", p=128)

    with tile.TileContext(nc) as tc:
        with tc.tile_pool(name="persist", bufs=1) as P:
            x2T = P.tile([128, DT, TQ], BF16)
            bvb = P.tile([128, D], BF16)
            r_row = P.tile([1, TB], F32)
            rt = P.tile([128, NTT], F32)
            ones = P.tile([128, 1], F32R)
            eps_t = P.tile([1, 1], F32)
            id11 = P.tile([1, 1], F32)
            bq_t = P.tile([128, DT], F32)
            bk_t = P.tile([128, DT], F32)
            pb_t = P.tile([128, DT], F32)
            b2_t = P.tile([128, DT], F32)
            b1_t = P.tile([128, FT], F32)

            ones_f = P.tile([128, 1], F32)
            ones_b = P.tile([128, 1], BF16)
            nc.vector.memset(ones_f[:], 1.0)
            nc.vector.tensor_copy(ones[:], ones_f[:])
            nc.vector.tensor_copy(ones_b[:], ones_f[:])
            nc.vector.memset(eps_t[:], EPS)
            nc.vector.memset(id11[:], 1.0)
            nc.scalar.dma_start(bq_t[:], bq.ap())
            nc.scalar.dma_start(bk_t[:], bk.ap())
            nc.scalar.dma_start(pb_t[:], pb.ap())
            nc.scalar.dma_start(b2_t[:], b2.ap())
            nc.scalar.dma_start(b1_t[:], b1.ap())

            # stats over the feature (partition) axis + centering, feature-major
            def stats_center(xsrc, W, r_slice, mu_bc, r_bc, dst, sqp, stp):
                sum_ps = stp.tile([1, W], F32, tag="sum_ps")
                sq_ps = stp.tile([1, W], F32, tag="sq_ps")
                for k in range(DT):
                    nc.tensor.matmul(sum_ps[:], ones_b[:], xsrc[:, k, :],
                                     start=(k == 0), stop=(k == DT - 1))
                for k in range(DT):
                    sq = sqp.tile([128, W], BF16, tag="sq")
                    nc.scalar.square(sq[:], xsrc[:, k, :])
                    nc.tensor.matmul(sq_ps[:], ones_b[:], sq[:],
                                     start=(k == 0), stop=(k == DT - 1))
                mu_r = sqp.tile([1, W], F32, tag="mu_r", bufs=1)
                nc.vector.tensor_scalar(mu_r[:], sum_ps[:], 1.0 / D, None, ALU.mult)
                musq = sqp.tile([1, W], F32, tag="musq", bufs=1)
                nc.vector.tensor_tensor(musq[:], mu_r[:], mu_r[:], ALU.mult)
                var = sqp.tile([1, W], F32, tag="var", bufs=1)
                nc.vector.scalar_tensor_tensor(var[:], sq_ps[:], 1.0 / D, musq[:],
                                               ALU.mult, ALU.subtract)
                std = sqp.tile([1, W], F32, tag="std", bufs=1)
                nc.scalar.activation(std[:], var[:], AF.Sqrt, bias=eps_t[:])
                nc.vector.reciprocal(r_slice, std[:])
                nc.gpsimd.partition_broadcast(mu_bc[:], mu_r[:])
                if r_bc is not None:
                    nc.gpsimd.partition_broadcast(r_bc[:], r_slice)
                # split centering DVE/Pool to halve the serial chain
                for k in range(DT):
                    eng = nc.vector if k % 3 else nc.gpsimd
                    eng.tensor_tensor(dst[:, k, :], xsrc[:, k, :], mu_bc[:],
                                      ALU.subtract)

            with tc.tile_pool(name="kvres", bufs=1) as KV:
                kT = KV.tile([128, DT, TB], BF16)
                V = KV.tile([128, NTT, c.NH * 65], BF16)
                qT = KV.tile([128, DT, TQ], BF16)
                xt = KV.tile([128, DT, TB], BF16)
                vone = V[:].rearrange("p t (h c) -> p t h c", c=65)

                # ----- Phase X+K fused: stream/center x, K per ready 512-block -----
                with tc.tile_pool(name="phx", bufs=1) as XP:
                    wq_t = XP.tile([128, DT, D], BF16)
                    with tc.tile_pool(name="phxs", bufs=2) as PX, \
                         tc.tile_pool(name="phxs_ps", bufs=2, space="PSUM") as PXP:
                        wk_t = PX.tile([128, DT, D], BF16, bufs=1)

                        # software pipeline: block i does PE sums + Act squares;
                        # block i-1 does sq-matmuls + finalize + centering, so
                        # PE never waits on the Act/DVE stats chain.
                        def finish(pv):
                            xi, off, xb, sum_ps, sq = pv
                            sq_ps = PXP.tile([1, XW], F32, tag="sq_ps")
                            for k in range(DT):
                                nc.tensor.matmul(sq_ps[:], ones_b[:],
                                                 sq[:, k, :],
                                                 start=(k == 0),
                                                 stop=(k == DT - 1))
                            mu_r = PX.tile([1, XW], F32, tag="mu_r")
                            nc.vector.tensor_scalar(mu_r[:], sum_ps[:], 1.0 / D,
                                                    None, ALU.mult)
                            musq = PX.tile([1, XW], F32, tag="musq")
                            nc.vector.tensor_tensor(musq[:], mu_r[:], mu_r[:],
                                                    ALU.mult)
                            var = PX.tile([1, XW], F32, tag="var")
                            nc.vector.scalar_tensor_tensor(
                                var[:], sq_ps[:], 1.0 / D, musq[:],
                                ALU.mult, ALU.subtract)
                            std = PX.tile([1, XW], F32, tag="std")
                            nc.scalar.activation(std[:], var[:], AF.Sqrt,
                                                 bias=eps_t[:])
                            nc.vector.reciprocal(r_row[0:1, off:off + XW],
                                                 std[:])
                            mu_bc = PX.tile([128, XW], F32, tag="mu_bc")
                            nc.gpsimd.partition_broadcast(mu_bc[:], mu_r[:])
                            for k in range(DT):
                                nc.vector.tensor_tensor(
                                    xt[:, k, off:off + XW], xb[:, k, :],
                                    mu_bc[:], ALU.subtract)

                        def do_k(b4):
                            off4 = b4 * 512
                            rb4 = PX.tile([128, 512], F32, tag="rb4", bufs=1)
                            nc.gpsimd.partition_broadcast(
                                rb4[:], r_row[0:1, off4:off4 + 512])
                            for m in range(DT):
                                ps = PXP.tile([128, 512], F32, tag="kps")
                                for k in range(DT):
                                    nc.tensor.matmul(
                                        ps[:], wk_t[:, k, ts(m, 128)],
                                        xt[:, k, off4:off4 + 512],
                                        start=(k == 0), stop=(k == DT - 1))
                                ev = PX.tile([128, 512], F32, tag="kev",
                                             bufs=2)
                                nc.vector.tensor_tensor(ev[:], ps[:], rb4[:],
                                                        ALU.mult)
                                nc.scalar.activation(
                                    kT[:, m, off4:off4 + 512], ev[:],
                                    AF.Identity, bias=bk_t[:, m:m + 1])
                            # rt transposes after K: stats chain long done
                            for tt in range(4):
                                g = b4 * 4 + tt
                                rt_ps = PXP.tile([128, 1], F32, tag="rt_ps")
                                nc.tensor.transpose(
                                    rt_ps[:],
                                    r_row[0:1, g * 128:(g + 1) * 128],
                                    id11[:])
                                nc.vector.tensor_copy(rt[:, g:g + 1],
                                                      rt_ps[:])

                        prev = None
                        for xi in range(NXB):
                            off = xi * XW
                            xb = PX.tile([128, DT, XW], F32R, tag="xb", bufs=2)
                            nc.sync.dma_start(xb[:],
                                              dram3(xT)[:, :, off:off + XW])
                            if xi == 1:
                                for kk in range(4):
                                    nc.sync.dma_start(
                                        wk_t[:, 2 * kk:2 * kk + 2, :],
                                        wk.ap()[:, 2 * kk:2 * kk + 2, :])
                            if xi == 2:
                                nc.sync.dma_start(wq_t[:], wq.ap())
                            sum_ps = PXP.tile([1, XW], F32, tag="sum_ps")
                            for k in range(DT):
                                nc.tensor.matmul(sum_ps[:], ones[:], xb[:, k, :],
                                                 start=(k == 0),
                                                 stop=(k == DT - 1))
                            sq = PX.tile([128, DT, XW], BF16, tag="sq", bufs=2)
                            nc.scalar.square(sq[:, 0:DT // 2, :],
                                             xb[:, 0:DT // 2, :])
                            nc.scalar.square(sq[:, DT // 2:DT, :],
                                             xb[:, DT // 2:DT, :])
                            if prev is not None:
                                finish(prev)
                                if xi >= 2 and xi % 2 == 0:
                                    do_k(xi // 2 - 1)
                            prev = (xi, off, xb, sum_ps, sq)
                        finish(prev)
                        do_k(NBLK - 1)

                    nc.vector.memset(vone[:, :, :, 64:65], 1.0)
                    bv_r = XP.tile([1, D], BF16)
                    nc.scalar.dma_start(bv_r[:], bv.ap())
                    nc.gpsimd.partition_broadcast(bvb[:], bv_r[:])

                    # ---------------- Phase Q ----------------
                    # q tiles sit at static positions {3,7,11,15}: reuse the
                    # centered xt and r_row stats from the X phase.
                    with tc.tile_pool(name="phq", bufs=2) as PQ, \
                         tc.tile_pool(name="phq_ps", bufs=2, space="PSUM") as PQP:
                        rq_r = PQ.tile([1, TQ], F32, bufs=1)
                        nc.vector.tensor_copy(
                            rq_r[:],
                            r_row[0:1, :].rearrange("o (g w) -> o g w", w=512)
                            [:, :, 384:512])
                        rq_bc = PQ.tile([128, TQ], F32, bufs=1)
                        nc.gpsimd.partition_broadcast(rq_bc[:], rq_r[:])
                        xt_q = xt[:].rearrange("p d (g w) -> p d g w", w=512)[
                            :, :, :, 384:512]
                        for m in range(DT):
                            ps = PQP.tile([128, TQ], F32, tag="qps")
                            for k in range(DT):
                                nc.tensor.matmul(
                                    ps[:].rearrange("p (g w) -> p g w", w=128),
                                    wq_t[:, k, ts(m, 128)], xt_q[:, k],
                                    start=(k == 0), stop=(k == DT - 1))
                            ev = PQ.tile([128, TQ], F32, tag="qev", bufs=2)
                            nc.vector.tensor_tensor(ev[:], ps[:], rq_bc[:],
                                                    ALU.mult)
                            nc.scalar.activation(qT[:, m, :], ev[:],
                                                 AF.Identity,
                                                 bias=bq_t[:, m:m + 1])

                # ------------- Phase attention (V folded into hp 0) -------------
                # php wraps pha so proj's weights/residual stream in during
                # attention (distinct SBUF -> no WAR serialization).
                with tc.tile_pool(name="php", bufs=1) as PP:
                  pw_t = PP.tile([128, DT, D], BF16)
                  nc.sync.dma_start(pw_t[:], pw.ap())
                  xq2 = PP.tile([128, DT, TQ], F32R)
                  nc.scalar.dma_start(xq2[:], dram3(xqT))
                  # per-k tiles so proj passes only dep on the heads they read
                  aoT = [PP.tile([128, TQ], BF16, name=f"aoT{k}")
                         for k in range(DT)]
                  with tc.tile_pool(name="pha", bufs=2) as PA:
                    mk = PA.tile([128, NTT, 128], BF16, bufs=1)
                    nc.scalar.dma_start(mk[:], maskM.ap())

                    def do_v(g):
                        for n in range(c.NVB):
                            ps = PPPP.tile([128, c.VN], F32, tag="ppool")
                            for k in range(DT):
                                nc.tensor.matmul(ps[:], xt[:, k, ts(g, 128)],
                                                 wvh[:, n, k, :],
                                                 start=(k == 0),
                                                 stop=(k == DT - 1))
                            dst = vone[:, g, n * c.HPV:(n + 1) * c.HPV, 0:64]
                            nc.vector.scalar_tensor_tensor(
                                dst, ps[:].rearrange("p (h c) -> p h c", c=64),
                                rt[:, g:g + 1],
                                bvb[:, ts(n, c.VN)].rearrange(
                                    "p (h c) -> p h c", c=64),
                                ALU.mult, ALU.add)

                    def proj_kpair(ms, k0, first):
                        # two proj k-tiles for m in ms, accumulated into part
                        for m in ms:
                            ps = PPPP.tile([128, TQ], F32, tag="ppool",
                                           name=f"pkp{k0}_{m}")
                            for k in (k0, k0 + 1):
                                nc.tensor.matmul(ps[:], pw_t[:, k, ts(m, 128)],
                                                 aoT[k][:], start=(k == k0),
                                                 stop=(k == k0 + 1))
                            if first:
                                nc.vector.tensor_copy(part[:, m, :], ps[:])
                            else:
                                nc.vector.tensor_tensor(part[:, m, :],
                                                        part[:, m, :], ps[:],
                                                        ALU.add)

                    def do_hp(hp):
                        # [65, TQ] f32 = one 2KB PSUM bank per half: one
                        # accumulation group each; truncated-width matmuls
                        # accumulate sub-columns, single stop at the last
                        # key tile closes the bank.
                        av0 = PAVP.tile([65, TQ], F32, tag="av0", bufs=1)
                        av1 = PAVP.tile([65, TQ], F32, tag="av1", bufs=1)
                        h0, h1 = 2 * hp, 2 * hp + 1
                        pq = []

                        def do_av(tk, p0, p1):
                            qlo = (tk // 4) * 128
                            for hh, av, pp in ((h0, av0, p0), (h1, av1, p1)):
                                nc.tensor.matmul(
                                    av[:, qlo:TQ],
                                    V[:, tk, hh * 65:(hh + 1) * 65],
                                    pp, start=(tk == 0),
                                    stop=(tk == NTT - 1))

                        # same-width key tiles grouped into one PSUM tile so
                        # exp and mask batch into single instructions
                        groups = [[0], [1], [2], [3], [4], [5], [6], [7],
                                  [8, 9], [10, 11], [12, 13, 14, 15]]
                        for grp in groups:
                            if hp == 0:
                                for tk in grp:
                                    do_v(tk)
                            ng = len(grp)
                            qlo = (grp[0] // 4) * 128
                            w = TQ - qlo
                            s01 = PAP.tile([128, 2, TQ], F32, tag="s01",
                                           bufs=2)
                            p01 = PA.tile([128, 2, TQ], BF16, tag="p01",
                                          bufs=4)
                            if ng == 1:
                                sv = s01[:].unsqueeze(2)[:, :, :, 0:w]
                                pv = p01[:].unsqueeze(2)[:, :, :, 0:w]
                            else:
                                sv = s01[:].rearrange("p h (g q) -> p h g q",
                                                      q=w)
                                pv = p01[:].rearrange("p h (g q) -> p h g q",
                                                      q=w)
                            for gi, tk in enumerate(grp):
                                nc.tensor.matmul(sv[:, 0, gi, :],
                                                 kT[0:64, hp, ts(tk, 128)],
                                                 qT[0:64, hp, qlo:TQ],
                                                 start=True, stop=True)
                                nc.tensor.matmul(sv[:, 1, gi, :],
                                                 kT[64:128, hp, ts(tk, 128)],
                                                 qT[64:128, hp, qlo:TQ],
                                                 start=True, stop=True)
                            nc.scalar.activation(p01[:, :, 0:ng * w],
                                                 s01[:, :, 0:ng * w], AF.Exp)
                            mkb = mk[:, grp[0]:grp[0] + ng, :].unsqueeze(
                                1).to_broadcast((128, 2, ng, 128))
                            nc.vector.tensor_tensor(pv[:, :, :, 0:128],
                                                    pv[:, :, :, 0:128], mkb,
                                                    ALU.mult)
                            for gi, tk in enumerate(grp):
                                pq.append((tk, pv[:, 0, gi, :],
                                           pv[:, 1, gi, :]))
                                if len(pq) > 2:
                                    do_av(*pq.pop(0))
                        while pq:
                            do_av(*pq.pop(0))
                        recs, rbcs = [], []
                        for av, half in ((av0, 0), (av1, 1)):
                            rec = PA.tile([1, TQ], F32, tag="rec", bufs=2)
                            nc.vector.reciprocal(rec[:], av[64:65, :])
                            recs.append(rec)
                        for half in (0, 1):
                            rbc = PA.tile([64, TQ], F32, tag="rbc", bufs=2)
                            nc.gpsimd.partition_broadcast(rbc[:], recs[half][:],
                                                          channels=64)
                            rbcs.append(rbc)
                        for av, half in ((av0, 0), (av1, 1)):
                            nc.vector.tensor_tensor(
                                aoT[hp][64 * half:64 * (half + 1), :],
                                av[0:64, :], rbcs[half][:], ALU.mult)

                    with tc.tile_pool(name="pha_ps", bufs=2, space="PSUM") as PAP, \
                         tc.tile_pool(name="pav_ps", bufs=1, space="PSUM") as PAVP, \
                         tc.tile_pool(name="ppp_ps", bufs=2, space="PSUM") as PPPP:
                        with tc.tile_pool(name="phv", bufs=1) as PV:
                            wvh = PV.tile([128, c.NVB, DT, c.VN], BF16)
                            for n in range(c.NVB):
                                nc.sync.dma_start(wvh[:, n], wv.ap()[n])
                            do_hp(0)
                        with tc.tile_pool(name="phpart", bufs=1) as PT:
                            part = PT.tile([128, DT, TQ], F32)
                            for hp in range(1, c.NH // 2):
                                do_hp(hp)
                                if hp >= 2:
                                    # fill Act-bound bubbles with proj k-pair
                                    # chains over finished aoT head tiles
                                    ms = (range(0, 4) if hp % 2 == 0
                                          else range(4, DT))
                                    proj_kpair(ms, 2 * ((hp - 2) // 2),
                                               hp < 4)

                            # ------------- proj tail: k tiles 6,7 -------------
                            # hoist k6 chains for m0/m1 ahead of the k7s
                            # (which wait on hp7's evac) to keep PE fed
                            def p2_chain(ps, m, ks):
                                for k in ks:
                                    nc.tensor.matmul(
                                        ps[:], pw_t[:, k, ts(m, 128)],
                                        aoT[k][:],
                                        start=(k == DT - 2),
                                        stop=(k == DT - 1))

                            pss = {}
                            for m in (0, 1):
                                pss[m] = PPPP.tile([128, TQ], F32,
                                                   tag="ppool",
                                                   name=f"p2ps{m}")
                                p2_chain(pss[m], m, [DT - 2])
                            for m in range(DT):
                                if m in pss:
                                    ps = pss[m]
                                    p2_chain(ps, m, [DT - 1])
                                else:
                                    ps = PPPP.tile([128, TQ], F32,
                                                   tag="ppool",
                                                   name=f"p2ps{m}")
                                    p2_chain(ps, m, [DT - 2, DT - 1])
                                nc.vector.scalar_tensor_tensor(
                                    x2T[:, m, :], ps[:], pb_t[:, m:m + 1],
                                    part[:, m, :], ALU.add, ALU.add)
                                nc.vector.tensor_tensor(
                                    x2T[:, m, :], x2T[:, m, :], xq2[:, m, :],
                                    ALU.add)

            # ---------------- Phase FFN ----------------
            with tc.tile_pool(name="phf", bufs=2) as PF, \
                 tc.tile_pool(name="phf_ps", bufs=2, space="PSUM") as PFP:
                h = PF.tile([128, FT, TQ], BF16, bufs=1)
                r2_r = PF.tile([1, TQ], F32, bufs=1)
                mu2_bc = PF.tile([128, TQ], F32, bufs=1)
                r2_bc = PF.tile([128, TQ], F32, bufs=1)
                w1c0 = PF.tile([128, DT, 256], BF16, bufs=1)
                nc.sync.dma_start(w1c0[:], w1.ap()[0])
                w2c0 = PF.tile([128, FT // 2, 256], BF16, tag="w2c", bufs=3)
                nc.scalar.dma_start(w2c0[:], w2.ap()[0])
                stats_center(x2T, TQ, r2_r[:], mu2_bc, r2_bc, x2T, PF, PFP)
                for mg in range(FT // 2):
                    if mg == 0:
                        w1c = w1c0
                    else:
                        w1c = PF.tile([128, DT, 256], BF16, tag="w1c", bufs=4)
                        nc.sync.dma_start(w1c[:], w1.ap()[mg])
                    for mi in range(2):
                        m = 2 * mg + mi
                        ps = PFP.tile([128, TQ], F32, tag="hps")
                        for k in range(DT):
                            nc.tensor.matmul(ps[:], w1c[:, k, ts(mi, 128)],
                                             x2T[:, k, :],
                                             start=(k == 0), stop=(k == DT - 1))
                        th = PF.tile([128, TQ], F32, tag="th", bufs=2)
                        nc.vector.tensor_tensor(th[:], ps[:], r2_bc[:], ALU.mult)
                        nc.scalar.activation(h[:, m, :], th[:], AF.Gelu,
                                             bias=b1_t[:, m:m + 1])
                for m in range(DT):
                    if m == 0:
                        w2c = w2c0
                    else:
                        w2c = PF.tile([128, FT // 2, 256], BF16, tag="w2c",
                                      bufs=3)
                        nc.sync.dma_start(w2c[:], w2.ap()[m])
                    ps = PFP.tile([128, TQ], F32, tag="ops")
                    for k in range(FT):
                        nc.tensor.matmul(
                            ps[:], w2c[:, k // 2, (k % 2) * 128:(k % 2) * 128 + 128],
                            h[:, k, :],
                            start=(k == 0), stop=(k == FT - 1))
                    t2 = PF.tile([128, TQ], F32, tag="t2", bufs=2)
                    nc.vector.scalar_tensor_tensor(t2[:], ps[:], b2_t[:, m:m + 1],
                                                   x2T[:, m, :], ALU.add, ALU.add)
                    ob = PF.tile([128, TQ], F32, tag="ob", bufs=2)
                    nc.vector.tensor_tensor(ob[:], t2[:], mu2_bc[:], ALU.add)
                    nc.sync.dma_start(
                        outT.ap().rearrange("(a p) t -> p a t", p=128)[:, m, :],
                        ob[:])
    nc.compile()
    return nc


_CACHE = {}


def _get_nc(c: CFG):
    key = (c.D, c.TB, c.TQ, c.NH, c.HFF)
    if key not in _CACHE:
        _CACHE[key] = build_nc(c)
    return _CACHE[key]


def core_perm(c: CFG, j: int):
    """Token-tile permutation for core j: within each group of 4 tiles the
    core's tile (index j in the group) moves to the group end."""
    tiles = []
    for a in range(c.TB // 512):
        grp = [4 * a + b for b in range(4) if b != j] + [4 * a + j]
        tiles.extend(grp)
    return tiles


def make_in_maps(c: CFG, x, mask, ln1_g, ln1_b, qkv_w, qkv_b, proj_w, proj_b,
                 ln2_g, ln2_b, w1, b1, w2, b2):
    D, TB, TQ, DT, FT = c.D, c.TB, c.TQ, c.DT, c.FT
    B = x.shape[0]
    ncg = TB // TQ  # query groups per batch

    f = np.float32
    bf = ml_dtypes.bfloat16
    g1 = ln1_g.astype(f)
    sc = 1.0 / np.sqrt(c.HD)
    DT, FT, VN, NVB = c.DT, c.FT, c.VN, c.NVB

    def tile_kp(w):  # [D, M] -> [128, DT, M] (partition-contiguous slabs)
        return np.ascontiguousarray(w.reshape(DT, 128, -1).transpose(1, 0, 2))

    wq_f = tile_kp((qkv_w[:, :D] * g1[:, None] * sc).astype(bf))
    wk_f = tile_kp((qkv_w[:, D:2 * D] * g1[:, None]).astype(bf))
    wv_b = (qkv_w[:, 2 * D:] * g1[:, None]).astype(bf)
    wv_f = np.ascontiguousarray(
        wv_b.reshape(DT, 128, NVB, VN).transpose(2, 1, 0, 3))
    bq_f = ((qkv_b[:D] + ln1_b @ qkv_w[:, :D]) * sc).astype(f)
    bk_f = (qkv_b[D:2 * D] + ln1_b @ qkv_w[:, D:2 * D]).astype(f)
    bv_f = (qkv_b[2 * D:] + ln1_b @ qkv_w[:, 2 * D:]).astype(f)
    w1g = (w1 * ln2_g.astype(f)[:, None]).astype(bf)
    w1_f = np.ascontiguousarray(
        w1g.reshape(DT, 128, FT // 2, 256).transpose(2, 1, 0, 3))
    b1_f = (b1 + ln2_b @ w1).astype(f)
    pw_f = tile_kp(np.asarray(proj_w, f).astype(bf))
    w2_f = np.ascontiguousarray(
        np.asarray(w2, f).astype(bf).reshape(FT, 128, DT, 128)
        .transpose(2, 1, 0, 3).reshape(DT, 128, FT // 2, 256))

    def btile(v, nt):
        return np.ascontiguousarray(v.reshape(nt, 128).T, f)

    z01 = np.asarray(mask[0, 0], f)  # [T,T] rows=queries, cols=keys

    shared = {
        "wq": wq_f, "wk": wk_f, "wv": wv_f, "pw": pw_f,
        "w1": w1_f, "w2": w2_f,
        "bq": btile(bq_f, DT), "bk": btile(bk_f, DT),
        "bv": np.ascontiguousarray(bv_f.reshape(1, D).astype(bf)),
        "pb": btile(proj_b.astype(f), DT),
        "b1": btile(b1_f, FT), "b2": btile(b2.astype(f), DT),
    }
    in_maps = []
    for core in range(B * ncg):
        b, j = core // ncg, core % ncg
        perm = core_perm(c, j)
        ptok = np.concatenate([np.arange(t * 128, (t + 1) * 128)
                               for t in perm])
        qtok = np.concatenate([np.arange((4 * i + j) * 128,
                                         (4 * i + j + 1) * 128)
                               for i in range(TQ // 128)])
        m = dict(shared)
        m["xT"] = np.ascontiguousarray(x[b][ptok].T, f)            # [D, TB]
        m["xqT"] = np.ascontiguousarray(x[b][qtok].T, f)           # [D, TQ]
        mm = np.empty((128, c.NTT, 128), np.float32)
        for kt in range(c.NTT):
            a = kt // 4
            gk, gq = perm[kt], 4 * a + j
            mm[:, kt, :] = z01[gq * 128:(gq + 1) * 128,
                               gk * 128:(gk + 1) * 128].T
        m["maskM"] = mm.astype(bf)
        in_maps.append(m)
    return in_maps


def assemble_out(c: CFG, results, B):
    ncg = c.TB // c.TQ
    out = np.empty((B, c.TB, c.D), np.float32)
    for core, res in enumerate(results):
        b, j = core // ncg, core % ncg
        o = res["outT"].T                                   # [TQ, D]
        for i in range(c.TQ // 128):
            t = 4 * i + j
            out[b, t * 128:(t + 1) * 128, :] = o[i * 128:(i + 1) * 128, :]
    return out


def kernel(x, mask, ln1_g, ln1_b, qkv_w, qkv_b, proj_w, proj_b,
           ln2_g, ln2_b, w1, b1, w2, b2):
    x = np.asarray(x, np.float32)
    c = CFG(D=x.shape[2], TB=x.shape[1], TQ=x.shape[1] // 4,
            NH=16, HD=64, HFF=4 * x.shape[2])
    nc = _get_nc(c)
    in_maps = make_in_maps(c, x, np.asarray(mask), *[np.asarray(a, np.float32)
                           for a in (ln1_g, ln1_b, qkv_w, qkv_b, proj_w, proj_b,
                                     ln2_g, ln2_b, w1, b1, w2, b2)])
    res = run_bass_kernel_spmd(nc, in_maps, core_ids=list(range(len(in_maps))))
    return assemble_out(c, res.results, x.shape[0])


if __name__ == "__main__":
    c = CFG()
    nc = build_nc(c)
    print("built ok")


# revision 64
# speedup vs baseline: 1.2688x; 1.0057x over previous
"""Trainium2 Bass kernel for a dense transformer block (pre-LN, causal attention, GELU FFN).

Sharding: 8 cores = 2 batches x 4 query-groups of 512 tokens, communication
free. Per-core the batch's token tiles are PERMUTED on host so that this
core's 4 query tiles sit at static positions {3,7,11,15} (within each group
of 4 tiles, the core's tile is moved to the group end, others stay in
ascending order). Keys stay causal-compatible: a query at position 4a+3 only
needs key positions 0..4a+3, so scores/AV run with causally truncated width
and only one diagonal 128x128 sub-tile per key tile needs mask application.
Q reuses the X-phase centered activations and row stats (no second LN pass).
All activations are feature-major [d, tokens]; LayerNorm is folded into
host-prepared weights plus per-token column stats applied at PSUM evacuation.
"""

import sys

sys.path.insert(0, "/opt/trn_rl_repo")

import numpy as np
import ml_dtypes

import concourse.bass as bass
import concourse.tile as tile
from concourse import bacc, mybir
from concourse.bass import ts
from concourse.bass_utils import run_bass_kernel_spmd

F32 = mybir.dt.float32
F32R = mybir.dt.float32r
BF16 = mybir.dt.bfloat16
AF = mybir.ActivationFunctionType
ALU = mybir.AluOpType

EPS = 1e-5


class CFG:
    def __init__(self, D=1024, TB=2048, TQ=512, NH=16, HD=64, HFF=4096):
        self.D, self.TB, self.TQ, self.NH, self.HD, self.HFF = D, TB, TQ, NH, HD, HFF
        self.DT = D // 128          # d_model tiles
        self.FT = HFF // 128        # ffn tiles
        self.NTT = TB // 128        # key token tiles
        self.NBLK = TB // 512       # 512-token kv blocks
        self.VN = min(512, D)       # V matmul free width
        self.NVB = D // self.VN     # V col blocks
        self.HPV = self.VN // HD    # heads per V col block
        assert NH == 2 * self.DT and HD == 64


def build_nc(c: CFG):
    nc = bacc.Bacc()
    D, TB, TQ, DT, FT, NTT, NBLK = c.D, c.TB, c.TQ, c.DT, c.FT, c.NTT, c.NBLK
    XW = 256                      # x-stream block width
    NXB = TB // XW
    NQT = TQ // 128               # query tiles per core

    xT = nc.dram_tensor("xT", [D, TB], F32R, kind="ExternalInput")
    xqT = nc.dram_tensor("xqT", [D, TQ], F32R, kind="ExternalInput")
    maskM = nc.dram_tensor("maskM", [128, NTT, 128], BF16, kind="ExternalInput")
    wq = nc.dram_tensor("wq", [128, DT, D], BF16, kind="ExternalInput")
    wk = nc.dram_tensor("wk", [128, DT, D], BF16, kind="ExternalInput")
    wv = nc.dram_tensor("wv", [c.NVB, 128, DT, c.VN], BF16, kind="ExternalInput")
    pw = nc.dram_tensor("pw", [128, DT, D], BF16, kind="ExternalInput")
    w1 = nc.dram_tensor("w1", [FT // 2, 128, DT, 256], BF16,
                        kind="ExternalInput")
    w2 = nc.dram_tensor("w2", [DT, 128, FT // 2, 256], BF16,
                        kind="ExternalInput")
    bq = nc.dram_tensor("bq", [128, DT], F32, kind="ExternalInput")
    bk = nc.dram_tensor("bk", [128, DT], F32, kind="ExternalInput")
    bv = nc.dram_tensor("bv", [1, D], BF16, kind="ExternalInput")
    pb = nc.dram_tensor("pb", [128, DT], F32, kind="ExternalInput")
    b1 = nc.dram_tensor("b1", [128, FT], F32, kind="ExternalInput")
    b2 = nc.dram_tensor("b2", [128, DT], F32, kind="ExternalInput")
    outT = nc.dram_tensor("outT", [D, TQ], F32, kind="ExternalOutput")

    def dram3(t):  # [ (a p), m ] -> [p, a, m]
        return t.ap().rearrange("(a p) m -> p a m", p=128)

    with tile.TileContext(nc) as tc:
        with tc.tile_pool(name="persist", bufs=1) as P:
            x2T = P.tile([128, DT, TQ], BF16)
            bvb = P.tile([128, D], BF16)
            r_row = P.tile([1, TB], F32)
            rt = P.tile([128, NTT], F32)
            ones = P.tile([128, 1], F32R)
            eps_t = P.tile([1, 1], F32)
            id11 = P.tile([1, 1], F32)
            mu2_r = P.tile([1, TQ], F32)
            r2_r = P.tile([1, TQ], F32)
            bq_t = P.tile([128, DT], F32)
            bk_t = P.tile([128, DT], F32)
            pb_t = P.tile([128, DT], F32)
            b2_t = P.tile([128, DT], F32)
            b1_t = P.tile([128, FT], F32)

            ones_f = P.tile([128, 1], F32)
            ones_b = P.tile([128, 1], BF16)
            nc.vector.memset(ones_f[:], 1.0)
            nc.vector.tensor_copy(ones[:], ones_f[:])
            nc.vector.tensor_copy(ones_b[:], ones_f[:])
            nc.vector.memset(eps_t[:], EPS)
            nc.vector.memset(id11[:], 1.0)
            nc.scalar.dma_start(bq_t[:], bq.ap())
            nc.scalar.dma_start(bk_t[:], bk.ap())
            nc.scalar.dma_start(pb_t[:], pb.ap())
            nc.scalar.dma_start(b2_t[:], b2.ap())
            nc.scalar.dma_start(b1_t[:], b1.ap())

            # stats over the feature (partition) axis + centering, feature-major
            def stats_center(xsrc, W, r_slice, mu_bc, r_bc, dst, sqp, stp):
                sum_ps = stp.tile([1, W], F32, tag="sum_ps")
                sq_ps = stp.tile([1, W], F32, tag="sq_ps")
                for k in range(DT):
                    nc.tensor.matmul(sum_ps[:], ones_b[:], xsrc[:, k, :],
                                     start=(k == 0), stop=(k == DT - 1))
                for k in range(DT):
                    sq = sqp.tile([128, W], BF16, tag="sq")
                    nc.scalar.square(sq[:], xsrc[:, k, :])
                    nc.tensor.matmul(sq_ps[:], ones_b[:], sq[:],
                                     start=(k == 0), stop=(k == DT - 1))
                mu_r = sqp.tile([1, W], F32, tag="mu_r", bufs=1)
                nc.vector.tensor_scalar(mu_r[:], sum_ps[:], 1.0 / D, None, ALU.mult)
                musq = sqp.tile([1, W], F32, tag="musq", bufs=1)
                nc.vector.tensor_tensor(musq[:], mu_r[:], mu_r[:], ALU.mult)
                var = sqp.tile([1, W], F32, tag="var", bufs=1)
                nc.vector.scalar_tensor_tensor(var[:], sq_ps[:], 1.0 / D, musq[:],
                                               ALU.mult, ALU.subtract)
                std = sqp.tile([1, W], F32, tag="std", bufs=1)
                nc.scalar.activation(std[:], var[:], AF.Sqrt, bias=eps_t[:])
                nc.vector.reciprocal(r_slice, std[:])
                nc.gpsimd.partition_broadcast(mu_bc[:], mu_r[:])
                if r_bc is not None:
                    nc.gpsimd.partition_broadcast(r_bc[:], r_slice)
                # split centering DVE/Pool to halve the serial chain
                for k in range(DT):
                    eng = nc.vector if k % 3 else nc.gpsimd
                    eng.tensor_tensor(dst[:, k, :], xsrc[:, k, :], mu_bc[:],
                                      ALU.subtract)

            with tc.tile_pool(name="kvres", bufs=1) as KV:
                kT = KV.tile([128, DT, TB], BF16)
                V = KV.tile([128, NTT, c.NH * 65], BF16)
                qT = KV.tile([128, DT, TQ], BF16)
                xt = KV.tile([128, DT, TB], BF16)
                vone = V[:].rearrange("p t (h c) -> p t h c", c=65)

                # ----- Phase X+K fused: stream/center x, K per ready 512-block -----
                with tc.tile_pool(name="phx", bufs=1) as XP:
                    wq_t = XP.tile([128, DT, D], BF16)
                    with tc.tile_pool(name="phxs", bufs=2) as PX, \
                         tc.tile_pool(name="phxs_ps", bufs=2, space="PSUM") as PXP:
                        wk_t = PX.tile([128, DT, D], BF16, bufs=1)

                        # software pipeline: block i does PE sums + Act squares;
                        # block i-1 does sq-matmuls + finalize + centering, so
                        # PE never waits on the Act/DVE stats chain.
                        def finish(pv):
                            xi, off, xb, sum_ps, sq = pv
                            sq_ps = PXP.tile([1, XW], F32, tag="sq_ps")
                            for k in range(DT):
                                nc.tensor.matmul(sq_ps[:], ones_b[:],
                                                 sq[:, k, :],
                                                 start=(k == 0),
                                                 stop=(k == DT - 1))
                            mu_r = PX.tile([1, XW], F32, tag="mu_r")
                            nc.vector.tensor_scalar(mu_r[:], sum_ps[:], 1.0 / D,
                                                    None, ALU.mult)
                            musq = PX.tile([1, XW], F32, tag="musq")
                            nc.vector.tensor_tensor(musq[:], mu_r[:], mu_r[:],
                                                    ALU.mult)
                            var = PX.tile([1, XW], F32, tag="var")
                            nc.vector.scalar_tensor_tensor(
                                var[:], sq_ps[:], 1.0 / D, musq[:],
                                ALU.mult, ALU.subtract)
                            std = PX.tile([1, XW], F32, tag="std")
                            nc.scalar.activation(std[:], var[:], AF.Sqrt,
                                                 bias=eps_t[:])
                            nc.vector.reciprocal(r_row[0:1, off:off + XW],
                                                 std[:])
                            mu_bc = PX.tile([128, XW], F32, tag="mu_bc")
                            nc.gpsimd.partition_broadcast(mu_bc[:], mu_r[:])
                            for k in range(DT):
                                nc.vector.tensor_tensor(
                                    xt[:, k, off:off + XW], xb[:, k, :],
                                    mu_bc[:], ALU.subtract)

                        def do_k(b4):
                            off4 = b4 * 512
                            rb4 = PX.tile([128, 512], F32, tag="rb4", bufs=1)
                            nc.gpsimd.partition_broadcast(
                                rb4[:], r_row[0:1, off4:off4 + 512])
                            for m in range(DT):
                                ps = PXP.tile([128, 512], F32, tag="kps")
                                for k in range(DT):
                                    nc.tensor.matmul(
                                        ps[:], wk_t[:, k, ts(m, 128)],
                                        xt[:, k, off4:off4 + 512],
                                        start=(k == 0), stop=(k == DT - 1))
                                ev = PX.tile([128, 512], F32, tag="kev",
                                             bufs=2)
                                nc.vector.tensor_tensor(ev[:], ps[:], rb4[:],
                                                        ALU.mult)
                                nc.scalar.activation(
                                    kT[:, m, off4:off4 + 512], ev[:],
                                    AF.Identity, bias=bk_t[:, m:m + 1])
                            # rt transposes after K: stats chain long done
                            for tt in range(4):
                                g = b4 * 4 + tt
                                rt_ps = PXP.tile([128, 1], F32, tag="rt_ps")
                                nc.tensor.transpose(
                                    rt_ps[:],
                                    r_row[0:1, g * 128:(g + 1) * 128],
                                    id11[:])
                                nc.vector.tensor_copy(rt[:, g:g + 1],
                                                      rt_ps[:])

                        prev = None
                        for xi in range(NXB):
                            off = xi * XW
                            xb = PX.tile([128, DT, XW], F32R, tag="xb", bufs=2)
                            nc.sync.dma_start(xb[:],
                                              dram3(xT)[:, :, off:off + XW])
                            if xi == 1:
                                for kk in range(4):
                                    nc.sync.dma_start(
                                        wk_t[:, 2 * kk:2 * kk + 2, :],
                                        wk.ap()[:, 2 * kk:2 * kk + 2, :])
                            if xi == 2:
                                nc.sync.dma_start(wq_t[:], wq.ap())
                            sum_ps = PXP.tile([1, XW], F32, tag="sum_ps")
                            for k in range(DT):
                                nc.tensor.matmul(sum_ps[:], ones[:], xb[:, k, :],
                                                 start=(k == 0),
                                                 stop=(k == DT - 1))
                            sq = PX.tile([128, DT, XW], BF16, tag="sq", bufs=2)
                            nc.scalar.square(sq[:, 0:DT // 2, :],
                                             xb[:, 0:DT // 2, :])
                            nc.scalar.square(sq[:, DT // 2:DT, :],
                                             xb[:, DT // 2:DT, :])
                            if prev is not None:
                                finish(prev)
                                if xi >= 2 and xi % 2 == 0:
                                    do_k(xi // 2 - 1)
                            prev = (xi, off, xb, sum_ps, sq)
                        finish(prev)
                        do_k(NBLK - 1)

                    nc.vector.memset(vone[:, :, :, 64:65], 1.0)
                    bv_r = XP.tile([1, D], BF16)
                    nc.scalar.dma_start(bv_r[:], bv.ap())
                    nc.gpsimd.partition_broadcast(bvb[:], bv_r[:])

                    # ---------------- Phase Q ----------------
                    # q tiles sit at static positions {3,7,11,15}: reuse the
                    # centered xt and r_row stats from the X phase.
                    with tc.tile_pool(name="phq", bufs=2) as PQ, \
                         tc.tile_pool(name="phq_ps", bufs=2, space="PSUM") as PQP:
                        rq_r = PQ.tile([1, TQ], F32, bufs=1)
                        nc.vector.tensor_copy(
                            rq_r[:],
                            r_row[0:1, :].rearrange("o (g w) -> o g w", w=512)
                            [:, :, 384:512])
                        rq_bc = PQ.tile([128, TQ], F32, bufs=1)
                        nc.gpsimd.partition_broadcast(rq_bc[:], rq_r[:])
                        xt_q = xt[:].rearrange("p d (g w) -> p d g w", w=512)[
                            :, :, :, 384:512]
                        for m in range(DT):
                            ps = PQP.tile([128, TQ], F32, tag="qps")
                            for k in range(DT):
                                nc.tensor.matmul(
                                    ps[:].rearrange("p (g w) -> p g w", w=128),
                                    wq_t[:, k, ts(m, 128)], xt_q[:, k],
                                    start=(k == 0), stop=(k == DT - 1))
                            ev = PQ.tile([128, TQ], F32, tag="qev", bufs=2)
                            nc.vector.tensor_tensor(ev[:], ps[:], rq_bc[:],
                                                    ALU.mult)
                            nc.scalar.activation(qT[:, m, :], ev[:],
                                                 AF.Identity,
                                                 bias=bq_t[:, m:m + 1])

                # ------------- Phase attention (V folded into hp 0) -------------
                # php wraps pha so proj's weights/residual stream in during
                # attention (distinct SBUF -> no WAR serialization).
                with tc.tile_pool(name="php", bufs=1) as PP:
                  pw_t = PP.tile([128, DT, D], BF16)
                  nc.sync.dma_start(pw_t[:], pw.ap())
                  xq2 = PP.tile([128, DT, TQ], F32R)
                  nc.scalar.dma_start(xq2[:], dram3(xqT))
                  # per-k tiles so proj passes only dep on the heads they read
                  aoT = [PP.tile([128, TQ], BF16, name=f"aoT{k}")
                         for k in range(DT)]
                  with tc.tile_pool(name="pha", bufs=2) as PA:
                    mk = PA.tile([128, NTT, 128], BF16, bufs=1)
                    nc.scalar.dma_start(mk[:], maskM.ap())

                    def do_v(g):
                        for n in range(c.NVB):
                            ps = PPPP.tile([128, c.VN], F32, tag="ppool")
                            for k in range(DT):
                                nc.tensor.matmul(ps[:], xt[:, k, ts(g, 128)],
                                                 wvh[:, n, k, :],
                                                 start=(k == 0),
                                                 stop=(k == DT - 1))
                            dst = vone[:, g, n * c.HPV:(n + 1) * c.HPV, 0:64]
                            nc.vector.scalar_tensor_tensor(
                                dst, ps[:].rearrange("p (h c) -> p h c", c=64),
                                rt[:, g:g + 1],
                                bvb[:, ts(n, c.VN)].rearrange(
                                    "p (h c) -> p h c", c=64),
                                ALU.mult, ALU.add)

                    def proj_ks(ms, ks, first):
                        # proj k-tile chain for m in ms, accumulated into part
                        for m in ms:
                            ps = PPPP.tile([128, TQ], F32, tag="ppool",
                                           name=f"pkp{ks[0]}_{m}")
                            for k in ks:
                                nc.tensor.matmul(ps[:], pw_t[:, k, ts(m, 128)],
                                                 aoT[k][:], start=(k == ks[0]),
                                                 stop=(k == ks[-1]))
                            if first:
                                nc.vector.tensor_copy(part[:, m, :], ps[:])
                            else:
                                nc.vector.tensor_tensor(part[:, m, :],
                                                        part[:, m, :], ps[:],
                                                        ALU.add)

                    def do_hp(hp):
                        # [65, TQ] f32 = one 2KB PSUM bank per half: one
                        # accumulation group each; truncated-width matmuls
                        # accumulate sub-columns, single stop at the last
                        # key tile closes the bank.
                        av0 = PAVP.tile([65, TQ], F32, tag="av0", bufs=1)
                        av1 = PAVP.tile([65, TQ], F32, tag="av1", bufs=1)
                        h0, h1 = 2 * hp, 2 * hp + 1
                        pq = []

                        def do_av(tk, p0, p1):
                            qlo = (tk // 4) * 128
                            for hh, av, pp in ((h0, av0, p0), (h1, av1, p1)):
                                nc.tensor.matmul(
                                    av[:, qlo:TQ],
                                    V[:, tk, hh * 65:(hh + 1) * 65],
                                    pp, start=(tk == 0),
                                    stop=(tk == NTT - 1))

                        # same-width key tiles grouped into one PSUM tile so
                        # exp and mask batch into single instructions
                        groups = [[0], [1], [2], [3], [4], [5], [6], [7],
                                  [8, 9], [10, 11], [12, 13, 14, 15]]
                        for grp in groups:
                            if hp == 0:
                                for tk in grp:
                                    do_v(tk)
                            ng = len(grp)
                            qlo = (grp[0] // 4) * 128
                            w = TQ - qlo
                            s01 = PAP.tile([128, 2, TQ], F32, tag="s01",
                                           bufs=2)
                            p01 = PA.tile([128, 2, TQ], BF16, tag="p01",
                                          bufs=4)
                            if ng == 1:
                                sv = s01[:].unsqueeze(2)[:, :, :, 0:w]
                                pv = p01[:].unsqueeze(2)[:, :, :, 0:w]
                            else:
                                sv = s01[:].rearrange("p h (g q) -> p h g q",
                                                      q=w)
                                pv = p01[:].rearrange("p h (g q) -> p h g q",
                                                      q=w)
                            for gi, tk in enumerate(grp):
                                nc.tensor.matmul(sv[:, 0, gi, :],
                                                 kT[0:64, hp, ts(tk, 128)],
                                                 qT[0:64, hp, qlo:TQ],
                                                 start=True, stop=True)
                                nc.tensor.matmul(sv[:, 1, gi, :],
                                                 kT[64:128, hp, ts(tk, 128)],
                                                 qT[64:128, hp, qlo:TQ],
                                                 start=True, stop=True)
                            nc.scalar.activation(p01[:, :, 0:ng * w],
                                                 s01[:, :, 0:ng * w], AF.Exp)
                            mkb = mk[:, grp[0]:grp[0] + ng, :].unsqueeze(
                                1).to_broadcast((128, 2, ng, 128))
                            nc.vector.tensor_tensor(pv[:, :, :, 0:128],
                                                    pv[:, :, :, 0:128], mkb,
                                                    ALU.mult)
                            for gi, tk in enumerate(grp):
                                pq.append((tk, pv[:, 0, gi, :],
                                           pv[:, 1, gi, :]))
                                if len(pq) > 2:
                                    do_av(*pq.pop(0))
                        while pq:
                            do_av(*pq.pop(0))
                        recs, rbcs = [], []
                        for av, half in ((av0, 0), (av1, 1)):
                            rec = PA.tile([1, TQ], F32, tag="rec", bufs=2)
                            nc.vector.reciprocal(rec[:], av[64:65, :])
                            recs.append(rec)
                        for half in (0, 1):
                            rbc = PA.tile([64, TQ], F32, tag="rbc", bufs=2)
                            nc.gpsimd.partition_broadcast(rbc[:], recs[half][:],
                                                          channels=64)
                            rbcs.append(rbc)
                        for av, half in ((av0, 0), (av1, 1)):
                            nc.vector.tensor_tensor(
                                aoT[hp][64 * half:64 * (half + 1), :],
                                av[0:64, :], rbcs[half][:], ALU.mult)

                    with tc.tile_pool(name="pha_ps", bufs=2, space="PSUM") as PAP, \
                         tc.tile_pool(name="pav_ps", bufs=1, space="PSUM") as PAVP, \
                         tc.tile_pool(name="ppp_ps", bufs=2, space="PSUM") as PPPP:
                        with tc.tile_pool(name="phv", bufs=1) as PV:
                            wvh = PV.tile([128, c.NVB, DT, c.VN], BF16)
                            for n in range(c.NVB):
                                nc.sync.dma_start(wvh[:, n], wv.ap()[n])
                            do_hp(0)
                        with tc.tile_pool(name="phpart", bufs=1) as PT:
                            part = PT.tile([128, DT, TQ], F32)
                            # proj filler at hp h's end over finished aoT
                            for hp in range(1, c.NH // 2):
                                do_hp(hp)
                                if hp >= 2:
                                    ms = (range(0, 4) if hp % 2 == 0
                                          else range(4, DT))
                                    k0 = 2 * ((hp - 2) // 2)
                                    proj_ks(ms, [k0, k0 + 1], hp < 4)

                            # ------------- proj tail: k tiles 6,7 -------------
                            # hoist k6 chains for m0/m1 ahead of the k7s
                            # (which wait on hp7's evac) to keep PE fed
                            def p2_chain(ps, m, ks):
                                for k in ks:
                                    nc.tensor.matmul(
                                        ps[:], pw_t[:, k, ts(m, 128)],
                                        aoT[k][:],
                                        start=(k == DT - 2),
                                        stop=(k == DT - 1))

                            pss = {}
                            for m in (0, 1):
                                pss[m] = PPPP.tile([128, TQ], F32,
                                                   tag="ppool",
                                                   name=f"p2ps{m}")
                                p2_chain(pss[m], m, [DT - 2])
                            sqs = []
                            for m in range(DT):
                                if m in pss:
                                    ps = pss[m]
                                    p2_chain(ps, m, [DT - 1])
                                else:
                                    ps = PPPP.tile([128, TQ], F32,
                                                   tag="ppool",
                                                   name=f"p2ps{m}")
                                    p2_chain(ps, m, [DT - 2, DT - 1])
                                nc.vector.scalar_tensor_tensor(
                                    x2T[:, m, :], ps[:], pb_t[:, m:m + 1],
                                    part[:, m, :], ALU.add, ALU.add)
                                nc.vector.tensor_tensor(
                                    x2T[:, m, :], x2T[:, m, :], xq2[:, m, :],
                                    ALU.add)
                                # pre-square for the FFN ln2 stats while the
                                # Act engine is otherwise idle
                                sq2 = PA.tile([128, 2, TQ], BF16, tag="p01",
                                              bufs=4, name=f"sq2_{m}")
                                nc.scalar.square(sq2[:, 0, :], x2T[:, m, :])
                                sqs.append(sq2)
                            # ln2 stats matmuls + scalar chain, still inside
                            # the attention scope (overlaps the proj tail)
                            sum2 = PAVP.tile([1, TQ], F32, tag="av0",
                                             name="sum2")
                            sq2p = PAVP.tile([1, TQ], F32, tag="av1",
                                             name="sq2p")
                            for k in range(DT):
                                nc.tensor.matmul(sum2[:], ones_b[:],
                                                 x2T[:, k, :],
                                                 start=(k == 0),
                                                 stop=(k == DT - 1))
                            for k in range(DT):
                                nc.tensor.matmul(sq2p[:], ones_b[:],
                                                 sqs[k][:, 0, :],
                                                 start=(k == 0),
                                                 stop=(k == DT - 1))
                            nc.vector.tensor_scalar(mu2_r[:], sum2[:],
                                                    1.0 / D, None, ALU.mult)
                            musq2 = PA.tile([1, TQ], F32, tag="rec",
                                            name="musq2")
                            nc.vector.tensor_tensor(musq2[:], mu2_r[:],
                                                    mu2_r[:], ALU.mult)
                            var2 = PA.tile([1, TQ], F32, tag="rec",
                                           name="var2")
                            nc.vector.scalar_tensor_tensor(
                                var2[:], sq2p[:], 1.0 / D, musq2[:],
                                ALU.mult, ALU.subtract)
                            std2 = PA.tile([1, TQ], F32, tag="rec",
                                           name="std2")
                            nc.scalar.activation(std2[:], var2[:], AF.Sqrt,
                                                 bias=eps_t[:])
                            nc.vector.reciprocal(r2_r[:], std2[:])

            # ---------------- Phase FFN ----------------
            with tc.tile_pool(name="phf", bufs=2) as PF, \
                 tc.tile_pool(name="phf_ps", bufs=2, space="PSUM") as PFP:
                h = PF.tile([128, FT, TQ], BF16, bufs=1)
                mu2_bc = PF.tile([128, TQ], F32, bufs=1)
                r2_bc = PF.tile([128, TQ], F32, bufs=1)
                w1c0 = PF.tile([128, DT, 256], BF16, bufs=1)
                nc.sync.dma_start(w1c0[:], w1.ap()[0])
                w2c0 = PF.tile([128, FT // 2, 256], BF16, tag="w2c", bufs=3)
                nc.scalar.dma_start(w2c0[:], w2.ap()[0])
                # stats were computed during the proj tail; broadcast, then
                # center per-k with the first FFN1 chains interleaved so PE
                # chases the centering instead of waiting for all of it
                nc.gpsimd.partition_broadcast(mu2_bc[:], mu2_r[:])
                nc.gpsimd.partition_broadcast(r2_bc[:], r2_r[:])
                ps_i0 = PFP.tile([128, TQ], F32, tag="hps")
                ps_i1 = PFP.tile([128, TQ], F32, tag="hps")
                for k in range(DT):
                    eng = nc.vector if k % 3 else nc.gpsimd
                    eng.tensor_tensor(x2T[:, k, :], x2T[:, k, :], mu2_bc[:],
                                      ALU.subtract)
                    nc.tensor.matmul(ps_i0[:], w1c0[:, k, ts(0, 128)],
                                     x2T[:, k, :],
                                     start=(k == 0), stop=(k == DT - 1))
                    nc.tensor.matmul(ps_i1[:], w1c0[:, k, ts(1, 128)],
                                     x2T[:, k, :],
                                     start=(k == 0), stop=(k == DT - 1))
                for m in (0, 1):
                    ps = ps_i0 if m == 0 else ps_i1
                    th = PF.tile([128, TQ], F32, tag="th", bufs=2,
                                 name=f"th_i{m}")
                    nc.vector.tensor_tensor(th[:], ps[:], r2_bc[:], ALU.mult)
                    nc.scalar.activation(h[:, m, :], th[:], AF.Gelu,
                                         bias=b1_t[:, m:m + 1])
                for mg in range(1, FT // 2):
                    w1c = PF.tile([128, DT, 256], BF16, tag="w1c", bufs=4)
                    nc.sync.dma_start(w1c[:], w1.ap()[mg])
                    for mi in range(2):
                        m = 2 * mg + mi
                        ps = PFP.tile([128, TQ], F32, tag="hps")
                        for k in range(DT):
                            nc.tensor.matmul(ps[:], w1c[:, k, ts(mi, 128)],
                                             x2T[:, k, :],
                                             start=(k == 0), stop=(k == DT - 1))
                        th = PF.tile([128, TQ], F32, tag="th", bufs=2)
                        nc.vector.tensor_tensor(th[:], ps[:], r2_bc[:], ALU.mult)
                        nc.scalar.activation(h[:, m, :], th[:], AF.Gelu,
                                             bias=b1_t[:, m:m + 1])
                for m in range(DT):
                    if m == 0:
                        w2c = w2c0
                    else:
                        w2c = PF.tile([128, FT // 2, 256], BF16, tag="w2c",
                                      bufs=3)
                        nc.sync.dma_start(w2c[:], w2.ap()[m])
                    ps = PFP.tile([128, TQ], F32, tag="ops")
                    for k in range(FT):
                        nc.tensor.matmul(
                            ps[:], w2c[:, k // 2, (k % 2) * 128:(k % 2) * 128 + 128],
                            h[:, k, :],
                            start=(k == 0), stop=(k == FT - 1))
                    t2 = PF.tile([128, TQ], F32, tag="t2", bufs=2)
                    nc.vector.scalar_tensor_tensor(t2[:], ps[:], b2_t[:, m:m + 1],
                                                   x2T[:, m, :], ALU.add, ALU.add)
                    ob = PF.tile([128, TQ], F32, tag="ob", bufs=2)
                    nc.vector.tensor_tensor(ob[:], t2[:], mu2_bc[:], ALU.add)
                    nc.sync.dma_start(
                        outT.ap().rearrange("(a p) t -> p a t", p=128)[:, m, :],
                        ob[:])
    nc.compile()
    return nc


_CACHE = {}


def _get_nc(c: CFG):
    key = (c.D, c.TB, c.TQ, c.NH, c.HFF)
    if key not in _CACHE:
        _CACHE[key] = build_nc(c)
    return _CACHE[key]


def core_perm(c: CFG, j: int):
    """Token-tile permutation for core j: within each group of 4 tiles the
    core's tile (index j in the group) moves to the group end."""
    tiles = []
    for a in range(c.TB // 512):
        grp = [4 * a + b for b in range(4) if b != j] + [4 * a + j]
        tiles.extend(grp)
    return tiles


def make_in_maps(c: CFG, x, mask, ln1_g, ln1_b, qkv_w, qkv_b, proj_w, proj_b,
                 ln2_g, ln2_b, w1, b1, w2, b2):
    D, TB, TQ, DT, FT = c.D, c.TB, c.TQ, c.DT, c.FT
    B = x.shape[0]
    ncg = TB // TQ  # query groups per batch

    f = np.float32
    bf = ml_dtypes.bfloat16
    g1 = ln1_g.astype(f)
    sc = 1.0 / np.sqrt(c.HD)
    DT, FT, VN, NVB = c.DT, c.FT, c.VN, c.NVB

    def tile_kp(w):  # [D, M] -> [128, DT, M] (partition-contiguous slabs)
        return np.ascontiguousarray(w.reshape(DT, 128, -1).transpose(1, 0, 2))

    wq_f = tile_kp((qkv_w[:, :D] * g1[:, None] * sc).astype(bf))
    wk_f = tile_kp((qkv_w[:, D:2 * D] * g1[:, None]).astype(bf))
    wv_b = (qkv_w[:, 2 * D:] * g1[:, None]).astype(bf)
    wv_f = np.ascontiguousarray(
        wv_b.reshape(DT, 128, NVB, VN).transpose(2, 1, 0, 3))
    bq_f = ((qkv_b[:D] + ln1_b @ qkv_w[:, :D]) * sc).astype(f)
    bk_f = (qkv_b[D:2 * D] + ln1_b @ qkv_w[:, D:2 * D]).astype(f)
    bv_f = (qkv_b[2 * D:] + ln1_b @ qkv_w[:, 2 * D:]).astype(f)
    w1g = (w1 * ln2_g.astype(f)[:, None]).astype(bf)
    w1_f = np.ascontiguousarray(
        w1g.reshape(DT, 128, FT // 2, 256).transpose(2, 1, 0, 3))
    b1_f = (b1 + ln2_b @ w1).astype(f)
    pw_f = tile_kp(np.asarray(proj_w, f).astype(bf))
    w2_f = np.ascontiguousarray(
        np.asarray(w2, f).astype(bf).reshape(FT, 128, DT, 128)
        .transpose(2, 1, 0, 3).reshape(DT, 128, FT // 2, 256))

    def btile(v, nt):
        return np.ascontiguousarray(v.reshape(nt, 128).T, f)

    z01 = np.asarray(mask[0, 0], f)  # [T,T] rows=queries, cols=keys

    shared = {
        "wq": wq_f, "wk": wk_f, "wv": wv_f, "pw": pw_f,
        "w1": w1_f, "w2": w2_f,
        "bq": btile(bq_f, DT), "bk": btile(bk_f, DT),
        "bv": np.ascontiguousarray(bv_f.reshape(1, D).astype(bf)),
        "pb": btile(proj_b.astype(f), DT),
        "b1": btile(b1_f, FT), "b2": btile(b2.astype(f), DT),
    }
    in_maps = []
    for core in range(B * ncg):
        b, j = core // ncg, core % ncg
        perm = core_perm(c, j)
        ptok = np.concatenate([np.arange(t * 128, (t + 1) * 128)
                               for t in perm])
        qtok = np.concatenate([np.arange((4 * i + j) * 128,
                                         (4 * i + j + 1) * 128)
                               for i in range(TQ // 128)])
        m = dict(shared)
        m["xT"] = np.ascontiguousarray(x[b][ptok].T, f)            # [D, TB]
        m["xqT"] = np.ascontiguousarray(x[b][qtok].T, f)           # [D, TQ]
        mm = np.empty((128, c.NTT, 128), np.float32)
        for kt in range(c.NTT):
            a = kt // 4
            gk, gq = perm[kt], 4 * a + j
            mm[:, kt, :] = z01[gq * 128:(gq + 1) * 128,
                               gk * 128:(gk + 1) * 128].T
        m["maskM"] = mm.astype(bf)
        in_maps.append(m)
    return in_maps


def assemble_out(c: CFG, results, B):
    ncg = c.TB // c.TQ
    out = np.empty((B, c.TB, c.D), np.float32)
    for core, res in enumerate(results):
        b, j = core // ncg, core % ncg
        o = res["outT"].T                                   # [TQ, D]
        for i in range(c.TQ // 128):
            t = 4 * i + j
            out[b, t * 128:(t + 1) * 128, :] = o[i * 128:(i + 1) * 128, :]
    return out


def kernel(x, mask, ln1_g, ln1_b, qkv_w, qkv_b, proj_w, proj_b,
           ln2_g, ln2_b, w1, b1, w2, b2):
    x = np.asarray(x, np.float32)
    c = CFG(D=x.shape[2], TB=x.shape[1], TQ=x.shape[1] // 4,
            NH=16, HD=64, HFF=4 * x.shape[2])
    nc = _get_nc(c)
    in_maps = make_in_maps(c, x, np.asarray(mask), *[np.asarray(a, np.float32)
                           for a in (ln1_g, ln1_b, qkv_w, qkv_b, proj_w, proj_b,
                                     ln2_g, ln2_b, w1, b1, w2, b2)])
    res = run_bass_kernel_spmd(nc, in_maps, core_ids=list(range(len(in_maps))))
    return assemble_out(c, res.results, x.shape[0])


if __name__ == "__main__":
    c = CFG()
    nc = build_nc(c)
    print("built ok")


# revision 73
# speedup vs baseline: 1.2714x; 1.0021x over previous
"""Trainium2 Bass kernel for a dense transformer block (pre-LN, causal attention, GELU FFN).

Sharding: 8 cores = 2 batches x 4 query-groups of 512 tokens, communication
free. Per-core the batch's token tiles are PERMUTED on host so that this
core's 4 query tiles sit at static positions {3,7,11,15} (within each group
of 4 tiles, the core's tile is moved to the group end, others stay in
ascending order). Keys stay causal-compatible: a query at position 4a+3 only
needs key positions 0..4a+3, so scores/AV run with causally truncated width
and only one diagonal 128x128 sub-tile per key tile needs mask application.
Q reuses the X-phase centered activations and row stats (no second LN pass).
All activations are feature-major [d, tokens]; LayerNorm is folded into
host-prepared weights plus per-token column stats applied at PSUM evacuation.
"""

import sys

sys.path.insert(0, "/opt/trn_rl_repo")

import numpy as np
import ml_dtypes

import concourse.bass as bass
import concourse.tile as tile
from concourse import bacc, mybir
from concourse.bass import ts
from concourse.bass_utils import run_bass_kernel_spmd

F32 = mybir.dt.float32
F32R = mybir.dt.float32r
BF16 = mybir.dt.bfloat16
AF = mybir.ActivationFunctionType
ALU = mybir.AluOpType

EPS = 1e-5


class CFG:
    def __init__(self, D=1024, TB=2048, TQ=512, NH=16, HD=64, HFF=4096):
        self.D, self.TB, self.TQ, self.NH, self.HD, self.HFF = D, TB, TQ, NH, HD, HFF
        self.DT = D // 128          # d_model tiles
        self.FT = HFF // 128        # ffn tiles
        self.NTT = TB // 128        # key token tiles
        self.NBLK = TB // 512       # 512-token kv blocks
        self.VN = min(512, D)       # V matmul free width
        self.NVB = D // self.VN     # V col blocks
        self.HPV = self.VN // HD    # heads per V col block
        assert NH == 2 * self.DT and HD == 64


def build_nc(c: CFG):
    nc = bacc.Bacc()
    D, TB, TQ, DT, FT, NTT, NBLK = c.D, c.TB, c.TQ, c.DT, c.FT, c.NTT, c.NBLK
    XW = 256                      # x-stream block width
    NXB = TB // XW
    NQT = TQ // 128               # query tiles per core

    xT = nc.dram_tensor("xT", [D, TB], F32R, kind="ExternalInput")
    xqT = nc.dram_tensor("xqT", [D, TQ], F32R, kind="ExternalInput")
    maskM = nc.dram_tensor("maskM", [128, NTT, 128], BF16, kind="ExternalInput")
    wq = nc.dram_tensor("wq", [128, DT, D], BF16, kind="ExternalInput")
    wk = nc.dram_tensor("wk", [128, DT, D], BF16, kind="ExternalInput")
    wv = nc.dram_tensor("wv", [c.NVB, 128, DT, c.VN], BF16, kind="ExternalInput")
    pw = nc.dram_tensor("pw", [128, DT, D], BF16, kind="ExternalInput")
    w1 = nc.dram_tensor("w1", [FT // 2, 128, DT, 256], BF16,
                        kind="ExternalInput")
    w2 = nc.dram_tensor("w2", [DT, 128, FT // 2, 256], BF16,
                        kind="ExternalInput")
    bq = nc.dram_tensor("bq", [128, DT], F32, kind="ExternalInput")
    bk = nc.dram_tensor("bk", [128, DT], F32, kind="ExternalInput")
    bv = nc.dram_tensor("bv", [1, D], BF16, kind="ExternalInput")
    pb = nc.dram_tensor("pb", [128, DT], F32, kind="ExternalInput")
    b1 = nc.dram_tensor("b1", [128, FT], F32, kind="ExternalInput")
    b2 = nc.dram_tensor("b2", [128, DT], F32, kind="ExternalInput")
    outT = nc.dram_tensor("outT", [D, TQ], F32, kind="ExternalOutput")

    def dram3(t):  # [ (a p), m ] -> [p, a, m]
        return t.ap().rearrange("(a p) m -> p a m", p=128)

    with tile.TileContext(nc) as tc:
        with tc.tile_pool(name="persist", bufs=1) as P:
            x2T = P.tile([128, DT, TQ], BF16)
            bvb = P.tile([128, D], BF16)
            r_row = P.tile([1, TB], F32)
            rt = P.tile([128, NTT], F32)
            ones = P.tile([128, 1], F32R)
            eps_t = P.tile([1, 1], F32)
            id11 = P.tile([1, 1], F32)
            mu2_r = P.tile([1, TQ], F32)
            r2_r = P.tile([1, TQ], F32)
            bq_t = P.tile([128, DT], F32)
            bk_t = P.tile([128, DT], F32)
            pb_t = P.tile([128, DT], F32)
            b2_t = P.tile([128, DT], F32)
            b1_t = P.tile([128, FT], F32)

            ones_f = P.tile([128, 1], F32)
            ones_b = P.tile([128, 1], BF16)
            nc.vector.memset(ones_f[:], 1.0)
            nc.vector.tensor_copy(ones[:], ones_f[:])
            nc.vector.tensor_copy(ones_b[:], ones_f[:])
            nc.vector.memset(eps_t[:], EPS)
            nc.vector.memset(id11[:], 1.0)
            warm = P.tile([1, 1], F32)
            nc.scalar.dma_start(bq_t[:], bq.ap())
            nc.scalar.dma_start(bk_t[:], bk.ap())
            nc.scalar.dma_start(pb_t[:], pb.ap())
            nc.scalar.dma_start(b2_t[:], b2.ap())
            nc.scalar.dma_start(b1_t[:], b1.ap())

            # stats over the feature (partition) axis + centering, feature-major
            def stats_center(xsrc, W, r_slice, mu_bc, r_bc, dst, sqp, stp):
                sum_ps = stp.tile([1, W], F32, tag="sum_ps")
                sq_ps = stp.tile([1, W], F32, tag="sq_ps")
                for k in range(DT):
                    nc.tensor.matmul(sum_ps[:], ones_b[:], xsrc[:, k, :],
                                     start=(k == 0), stop=(k == DT - 1))
                for k in range(DT):
                    sq = sqp.tile([128, W], BF16, tag="sq")
                    nc.scalar.square(sq[:], xsrc[:, k, :])
                    nc.tensor.matmul(sq_ps[:], ones_b[:], sq[:],
                                     start=(k == 0), stop=(k == DT - 1))
                mu_r = sqp.tile([1, W], F32, tag="mu_r", bufs=1)
                nc.vector.tensor_scalar(mu_r[:], sum_ps[:], 1.0 / D, None, ALU.mult)
                musq = sqp.tile([1, W], F32, tag="musq", bufs=1)
                nc.vector.tensor_tensor(musq[:], mu_r[:], mu_r[:], ALU.mult)
                var = sqp.tile([1, W], F32, tag="var", bufs=1)
                nc.vector.scalar_tensor_tensor(var[:], sq_ps[:], 1.0 / D, musq[:],
                                               ALU.mult, ALU.subtract)
                std = sqp.tile([1, W], F32, tag="std", bufs=1)
                nc.scalar.activation(std[:], var[:], AF.Sqrt, bias=eps_t[:])
                nc.vector.reciprocal(r_slice, std[:])
                nc.gpsimd.partition_broadcast(mu_bc[:], mu_r[:])
                if r_bc is not None:
                    nc.gpsimd.partition_broadcast(r_bc[:], r_slice)
                # split centering DVE/Pool to halve the serial chain
                for k in range(DT):
                    eng = nc.vector if k % 3 else nc.gpsimd
                    eng.tensor_tensor(dst[:, k, :], xsrc[:, k, :], mu_bc[:],
                                      ALU.subtract)

            with tc.tile_pool(name="kvres", bufs=1) as KV:
                kT = KV.tile([128, DT, TB], BF16)
                V = KV.tile([128, NTT, c.NH * 65], BF16)
                qT = KV.tile([128, DT, TQ], BF16)
                xt = KV.tile([128, DT, TB], BF16)
                vone = V[:].rearrange("p t (h c) -> p t h c", c=65)

                # ----- Phase X+K fused: stream/center x, K per ready 512-block -----
                with tc.tile_pool(name="phx", bufs=1) as XP:
                    wq_t = XP.tile([128, DT, D], BF16)
                    with tc.tile_pool(name="phxs", bufs=2) as PX, \
                         tc.tile_pool(name="phxs_ps", bufs=2, space="PSUM") as PXP:
                        wk_t = PX.tile([128, DT, D], BF16, bufs=1)

                        # software pipeline: block i does PE sums + Act squares;
                        # block i-1 does sq-matmuls + finalize + centering, so
                        # PE never waits on the Act/DVE stats chain.
                        def finish(pv):
                            xi, off, xb, sum_ps, sq = pv
                            sq_ps = PXP.tile([1, XW], F32, tag="sq_ps")
                            for k in range(DT):
                                nc.tensor.matmul(sq_ps[:], ones_b[:],
                                                 sq[:, k, :],
                                                 start=(k == 0),
                                                 stop=(k == DT - 1))
                            mu_r = PX.tile([1, XW], F32, tag="mu_r")
                            nc.vector.tensor_scalar(mu_r[:], sum_ps[:], 1.0 / D,
                                                    None, ALU.mult)
                            musq = PX.tile([1, XW], F32, tag="musq")
                            nc.vector.tensor_tensor(musq[:], mu_r[:], mu_r[:],
                                                    ALU.mult)
                            var = PX.tile([1, XW], F32, tag="var")
                            nc.vector.scalar_tensor_tensor(
                                var[:], sq_ps[:], 1.0 / D, musq[:],
                                ALU.mult, ALU.subtract)
                            std = PX.tile([1, XW], F32, tag="std")
                            nc.scalar.activation(std[:], var[:], AF.Sqrt,
                                                 bias=eps_t[:])
                            nc.vector.reciprocal(r_row[0:1, off:off + XW],
                                                 std[:])
                            mu_bc = PX.tile([128, XW], F32, tag="mu_bc")
                            nc.gpsimd.partition_broadcast(mu_bc[:], mu_r[:])
                            for k in range(DT):
                                nc.vector.tensor_tensor(
                                    xt[:, k, off:off + XW], xb[:, k, :],
                                    mu_bc[:], ALU.subtract)

                        def do_k(b4):
                            off4 = b4 * 512
                            rb4 = PX.tile([128, 512], F32, tag="rb4", bufs=1)
                            nc.gpsimd.partition_broadcast(
                                rb4[:], r_row[0:1, off4:off4 + 512])
                            for m in range(DT):
                                ps = PXP.tile([128, 512], F32, tag="kps")
                                for k in range(DT):
                                    nc.tensor.matmul(
                                        ps[:], wk_t[:, k, ts(m, 128)],
                                        xt[:, k, off4:off4 + 512],
                                        start=(k == 0), stop=(k == DT - 1))
                                ev = PX.tile([128, 512], F32, tag="kev",
                                             bufs=2)
                                nc.vector.tensor_tensor(ev[:], ps[:], rb4[:],
                                                        ALU.mult)
                                nc.scalar.activation(
                                    kT[:, m, off4:off4 + 512], ev[:],
                                    AF.Identity, bias=bk_t[:, m:m + 1])
                            # rt transposes after K: stats chain long done
                            for tt in range(4):
                                g = b4 * 4 + tt
                                rt_ps = PXP.tile([128, 1], F32, tag="rt_ps")
                                nc.tensor.transpose(
                                    rt_ps[:],
                                    r_row[0:1, g * 128:(g + 1) * 128],
                                    id11[:])
                                nc.vector.tensor_copy(rt[:, g:g + 1],
                                                      rt_ps[:])

                        prev = None
                        for xi in range(NXB):
                            off = xi * XW
                            xb = PX.tile([128, DT, XW], F32R, tag="xb", bufs=2)
                            nc.sync.dma_start(xb[:],
                                              dram3(xT)[:, :, off:off + XW])
                            if xi == 1:
                                for kk in range(4):
                                    nc.sync.dma_start(
                                        wk_t[:, 2 * kk:2 * kk + 2, :],
                                        wk.ap()[:, 2 * kk:2 * kk + 2, :])
                            if xi == 2:
                                nc.sync.dma_start(wq_t[:], wq.ap())
                            sum_ps = PXP.tile([1, XW], F32, tag="sum_ps")
                            for k in range(DT):
                                nc.tensor.matmul(sum_ps[:], ones[:], xb[:, k, :],
                                                 start=(k == 0),
                                                 stop=(k == DT - 1))
                            sq = PX.tile([128, DT, XW], BF16, tag="sq", bufs=2)
                            nc.scalar.square(sq[:, 0:DT // 2, :],
                                             xb[:, 0:DT // 2, :])
                            nc.scalar.square(sq[:, DT // 2:DT, :],
                                             xb[:, DT // 2:DT, :])
                            if prev is not None:
                                finish(prev)
                                if xi >= 2 and xi % 2 == 0:
                                    do_k(xi // 2 - 1)
                            prev = (xi, off, xb, sum_ps, sq)
                        finish(prev)
                        do_k(NBLK - 1)

                    nc.vector.memset(vone[:, :, :, 64:65], 1.0)
                    bv_r = XP.tile([1, D], BF16)
                    nc.scalar.dma_start(bv_r[:], bv.ap())
                    nc.gpsimd.partition_broadcast(bvb[:], bv_r[:])

                    # ---------------- Phase Q ----------------
                    # q tiles sit at static positions {3,7,11,15}: reuse the
                    # centered xt and r_row stats from the X phase.
                    with tc.tile_pool(name="phq", bufs=2) as PQ, \
                         tc.tile_pool(name="phq_ps", bufs=2, space="PSUM") as PQP:
                        nc.scalar.activation(warm[:], eps_t[:], AF.Exp)
                        nc.scalar.activation(warm[:], eps_t[:], AF.Gelu,
                                             bias=eps_t[:])
                        rq_r = PQ.tile([1, TQ], F32, bufs=1)
                        nc.vector.tensor_copy(
                            rq_r[:],
                            r_row[0:1, :].rearrange("o (g w) -> o g w", w=512)
                            [:, :, 384:512])
                        rq_bc = PQ.tile([128, TQ], F32, bufs=1)
                        nc.gpsimd.partition_broadcast(rq_bc[:], rq_r[:])
                        xt_q = xt[:].rearrange("p d (g w) -> p d g w", w=512)[
                            :, :, :, 384:512]
                        for m in range(DT):
                            ps = PQP.tile([128, TQ], F32, tag="qps")
                            for k in range(DT):
                                nc.tensor.matmul(
                                    ps[:].rearrange("p (g w) -> p g w", w=128),
                                    wq_t[:, k, ts(m, 128)], xt_q[:, k],
                                    start=(k == 0), stop=(k == DT - 1))
                            ev = PQ.tile([128, TQ], F32, tag="qev", bufs=2)
                            nc.vector.tensor_tensor(ev[:], ps[:], rq_bc[:],
                                                    ALU.mult)
                            nc.scalar.activation(qT[:, m, :], ev[:],
                                                 AF.Identity,
                                                 bias=bq_t[:, m:m + 1])

                # ------------- Phase attention (V folded into hp 0) -------------
                # php wraps pha so proj's weights/residual stream in during
                # attention (distinct SBUF -> no WAR serialization).
                with tc.tile_pool(name="php", bufs=1) as PP:
                  pw_t = PP.tile([128, DT, D], BF16)
                  nc.sync.dma_start(pw_t[:], pw.ap())
                  xq2 = PP.tile([128, DT, TQ], F32R)
                  nc.scalar.dma_start(xq2[:], dram3(xqT))
                  # per-k tiles so proj passes only dep on the heads they read
                  aoT = [PP.tile([128, TQ], BF16, name=f"aoT{k}")
                         for k in range(DT)]
                  with tc.tile_pool(name="pha", bufs=2) as PA:
                    mk = PA.tile([128, NTT, 128], BF16, bufs=1)
                    nc.scalar.dma_start(mk[:], maskM.ap())

                    def do_v(g):
                        for n in range(c.NVB):
                            ps = PPPP.tile([128, c.VN], F32, tag="ppool")
                            for k in range(DT):
                                nc.tensor.matmul(ps[:], xt[:, k, ts(g, 128)],
                                                 wvh[:, n, k, :],
                                                 start=(k == 0),
                                                 stop=(k == DT - 1))
                            dst = vone[:, g, n * c.HPV:(n + 1) * c.HPV, 0:64]
                            nc.vector.scalar_tensor_tensor(
                                dst, ps[:].rearrange("p (h c) -> p h c", c=64),
                                rt[:, g:g + 1],
                                bvb[:, ts(n, c.VN)].rearrange(
                                    "p (h c) -> p h c", c=64),
                                ALU.mult, ALU.add)

                    def proj_ks(ms, ks, first):
                        # proj k-tile chain for m in ms, accumulated into part
                        for m in ms:
                            ps = PPPP.tile([128, TQ], F32, tag="ppool",
                                           name=f"pkp{ks[0]}_{m}")
                            for k in ks:
                                nc.tensor.matmul(ps[:], pw_t[:, k, ts(m, 128)],
                                                 aoT[k][:], start=(k == ks[0]),
                                                 stop=(k == ks[-1]))
                            if first:
                                nc.vector.scalar_tensor_tensor(
                                    part[:, m, :], ps[:], 1.0, xq2[:, m, :],
                                    ALU.mult, ALU.add)
                            else:
                                nc.vector.tensor_tensor(part[:, m, :],
                                                        part[:, m, :], ps[:],
                                                        ALU.add)

                    def do_hp(hp):
                        # [65, TQ] f32 = one 2KB PSUM bank per half: one
                        # accumulation group each; truncated-width matmuls
                        # accumulate sub-columns, single stop at the last
                        # key tile closes the bank.
                        av0 = PAVP.tile([65, TQ], F32, tag="av0", bufs=1)
                        av1 = PAVP.tile([65, TQ], F32, tag="av1", bufs=1)
                        h0, h1 = 2 * hp, 2 * hp + 1
                        pq = []

                        def do_av(tk, p0, p1):
                            qlo = (tk // 4) * 128
                            for hh, av, pp in ((h0, av0, p0), (h1, av1, p1)):
                                nc.tensor.matmul(
                                    av[:, qlo:TQ],
                                    V[:, tk, hh * 65:(hh + 1) * 65],
                                    pp, start=(tk == 0),
                                    stop=(tk == NTT - 1))

                        # same-width key tiles grouped into one PSUM tile so
                        # exp and mask batch into single instructions
                        groups = [[0], [1], [2], [3], [4], [5], [6], [7],
                                  [8, 9], [10, 11], [12, 13, 14, 15]]
                        for grp in groups:
                            if hp == 0:
                                for tk in grp:
                                    do_v(tk)
                            ng = len(grp)
                            qlo = (grp[0] // 4) * 128
                            w = TQ - qlo
                            s01 = PAP.tile([128, 2, TQ], F32, tag="s01",
                                           bufs=2)
                            p01 = PA.tile([128, 2, TQ], BF16, tag="p01",
                                          bufs=4)
                            if ng == 1:
                                sv = s01[:].unsqueeze(2)[:, :, :, 0:w]
                                pv = p01[:].unsqueeze(2)[:, :, :, 0:w]
                            else:
                                sv = s01[:].rearrange("p h (g q) -> p h g q",
                                                      q=w)
                                pv = p01[:].rearrange("p h (g q) -> p h g q",
                                                      q=w)
                            for gi, tk in enumerate(grp):
                                nc.tensor.matmul(sv[:, 0, gi, :],
                                                 kT[0:64, hp, ts(tk, 128)],
                                                 qT[0:64, hp, qlo:TQ],
                                                 start=True, stop=True)
                                nc.tensor.matmul(sv[:, 1, gi, :],
                                                 kT[64:128, hp, ts(tk, 128)],
                                                 qT[64:128, hp, qlo:TQ],
                                                 start=True, stop=True)
                            nc.scalar.activation(p01[:, :, 0:ng * w],
                                                 s01[:, :, 0:ng * w], AF.Exp)
                            mkb = mk[:, grp[0]:grp[0] + ng, :].unsqueeze(
                                1).to_broadcast((128, 2, ng, 128))
                            nc.vector.tensor_tensor(pv[:, :, :, 0:128],
                                                    pv[:, :, :, 0:128], mkb,
                                                    ALU.mult)
                            for gi, tk in enumerate(grp):
                                pq.append((tk, pv[:, 0, gi, :],
                                           pv[:, 1, gi, :]))
                                if len(pq) > 2:
                                    do_av(*pq.pop(0))
                        while pq:
                            do_av(*pq.pop(0))
                        recs, rbcs = [], []
                        for av, half in ((av0, 0), (av1, 1)):
                            rec = PA.tile([1, TQ], F32, tag="rec", bufs=2)
                            nc.vector.reciprocal(rec[:], av[64:65, :])
                            recs.append(rec)
                        for half in (0, 1):
                            rbc = PA.tile([64, TQ], F32, tag="rbc", bufs=2)
                            nc.gpsimd.partition_broadcast(rbc[:], recs[half][:],
                                                          channels=64)
                            rbcs.append(rbc)
                        for av, half in ((av0, 0), (av1, 1)):
                            nc.vector.tensor_tensor(
                                aoT[hp][64 * half:64 * (half + 1), :],
                                av[0:64, :], rbcs[half][:], ALU.mult)

                    with tc.tile_pool(name="pha_ps", bufs=2, space="PSUM") as PAP, \
                         tc.tile_pool(name="pav_ps", bufs=1, space="PSUM") as PAVP, \
                         tc.tile_pool(name="ppp_ps", bufs=2, space="PSUM") as PPPP:
                        with tc.tile_pool(name="phv", bufs=1) as PV:
                            wvh = PV.tile([128, c.NVB, DT, c.VN], BF16)
                            for n in range(c.NVB):
                                nc.sync.dma_start(wvh[:, n], wv.ap()[n])
                            do_hp(0)
                        with tc.tile_pool(name="phpart", bufs=1) as PT:
                            part = PT.tile([128, DT, TQ], F32)
                            # proj filler at hp h's end over finished aoT
                            for hp in range(1, c.NH // 2):
                                do_hp(hp)
                                if hp >= 2:
                                    ms = (range(0, 4) if hp % 2 == 0
                                          else range(4, DT))
                                    k0 = 2 * ((hp - 2) // 2)
                                    proj_ks(ms, [k0, k0 + 1], hp < 4)

                            # ------------- proj tail: k tiles 6,7 -------------
                            # hoist k6 chains for m0/m1 ahead of the k7s
                            # (which wait on hp7's evac) to keep PE fed
                            def p2_chain(ps, m, ks):
                                for k in ks:
                                    nc.tensor.matmul(
                                        ps[:], pw_t[:, k, ts(m, 128)],
                                        aoT[k][:],
                                        start=(k == DT - 2),
                                        stop=(k == DT - 1))

                            pss = {}
                            for m in (0, 1):
                                pss[m] = PPPP.tile([128, TQ], F32,
                                                   tag="ppool",
                                                   name=f"p2ps{m}")
                                p2_chain(pss[m], m, [DT - 2])
                            sqs = []
                            for m in range(DT):
                                if m in pss:
                                    ps = pss[m]
                                    p2_chain(ps, m, [DT - 1])
                                else:
                                    ps = PPPP.tile([128, TQ], F32,
                                                   tag="ppool",
                                                   name=f"p2ps{m}")
                                    p2_chain(ps, m, [DT - 2, DT - 1])
                                nc.vector.scalar_tensor_tensor(
                                    x2T[:, m, :], ps[:], pb_t[:, m:m + 1],
                                    part[:, m, :], ALU.add, ALU.add)
                                # pre-square for the FFN ln2 stats while the
                                # Act engine is otherwise idle
                                sq2 = PA.tile([128, 2, TQ], BF16, tag="p01",
                                              bufs=4, name=f"sq2_{m}")
                                nc.scalar.square(sq2[:, 0, :], x2T[:, m, :])
                                sqs.append(sq2)
                            # ln2 stats matmuls + scalar chain, still inside
                            # the attention scope (overlaps the proj tail)
                            sum2 = PAVP.tile([1, TQ], F32, tag="av0",
                                             name="sum2")
                            sq2p = PAVP.tile([1, TQ], F32, tag="av1",
                                             name="sq2p")
                            for k in range(DT):
                                nc.tensor.matmul(sum2[:], ones_b[:],
                                                 x2T[:, k, :],
                                                 start=(k == 0),
                                                 stop=(k == DT - 1))
                            for k in range(DT):
                                nc.tensor.matmul(sq2p[:], ones_b[:],
                                                 sqs[k][:, 0, :],
                                                 start=(k == 0),
                                                 stop=(k == DT - 1))
                            nc.vector.tensor_scalar(mu2_r[:], sum2[:],
                                                    1.0 / D, None, ALU.mult)
                            musq2 = PA.tile([1, TQ], F32, tag="rec",
                                            name="musq2")
                            nc.vector.tensor_tensor(musq2[:], mu2_r[:],
                                                    mu2_r[:], ALU.mult)
                            var2 = PA.tile([1, TQ], F32, tag="rec",
                                           name="var2")
                            nc.vector.scalar_tensor_tensor(
                                var2[:], sq2p[:], 1.0 / D, musq2[:],
                                ALU.mult, ALU.subtract)
                            std2 = PA.tile([1, TQ], F32, tag="rec",
                                           name="std2")
                            nc.scalar.activation(std2[:], var2[:], AF.Sqrt,
                                                 bias=eps_t[:])
                            nc.vector.reciprocal(r2_r[:], std2[:])

            # ---------------- Phase FFN ----------------
            with tc.tile_pool(name="phf", bufs=2) as PF, \
                 tc.tile_pool(name="phf_ps", bufs=2, space="PSUM") as PFP:
                h = PF.tile([128, FT, TQ], BF16, bufs=1)
                mu2_bc = PF.tile([128, TQ], F32, bufs=1)
                r2_bc = PF.tile([128, TQ], F32, bufs=1)
                w1c0 = PF.tile([128, DT, 256], BF16, bufs=1)
                nc.sync.dma_start(w1c0[:], w1.ap()[0])
                w2c0 = PF.tile([128, FT // 2, 256], BF16, tag="w2c", bufs=3)
                nc.scalar.dma_start(w2c0[:], w2.ap()[0])
                # stats were computed during the proj tail; broadcast, then
                # center per-k with the first FFN1 chains interleaved so PE
                # chases the centering instead of waiting for all of it
                nc.gpsimd.partition_broadcast(mu2_bc[:], mu2_r[:])
                nc.gpsimd.partition_broadcast(r2_bc[:], r2_r[:])
                ps_i0 = PFP.tile([128, TQ], F32, tag="hps")
                ps_i1 = PFP.tile([128, TQ], F32, tag="hps")
                for k in range(DT):
                    eng = nc.gpsimd if k % 3 == 2 else nc.vector
                    eng.tensor_tensor(x2T[:, k, :], x2T[:, k, :], mu2_bc[:],
                                      ALU.subtract)
                    nc.tensor.matmul(ps_i0[:], w1c0[:, k, ts(0, 128)],
                                     x2T[:, k, :],
                                     start=(k == 0), stop=(k == DT - 1))
                    nc.tensor.matmul(ps_i1[:], w1c0[:, k, ts(1, 128)],
                                     x2T[:, k, :],
                                     start=(k == 0), stop=(k == DT - 1))
                for m in (0, 1):
                    ps = ps_i0 if m == 0 else ps_i1
                    th = PF.tile([128, TQ], F32, tag="th", bufs=2,
                                 name=f"th_i{m}")
                    nc.vector.tensor_tensor(th[:], ps[:], r2_bc[:], ALU.mult)
                    nc.scalar.activation(h[:, m, :], th[:], AF.Gelu,
                                         bias=b1_t[:, m:m + 1])
                for mg in range(1, FT // 2):
                    w1c = PF.tile([128, DT, 256], BF16, tag="w1c", bufs=4)
                    nc.sync.dma_start(w1c[:], w1.ap()[mg])
                    for mi in range(2):
                        m = 2 * mg + mi
                        ps = PFP.tile([128, TQ], F32, tag="hps")
                        for k in range(DT):
                            nc.tensor.matmul(ps[:], w1c[:, k, ts(mi, 128)],
                                             x2T[:, k, :],
                                             start=(k == 0), stop=(k == DT - 1))
                        th = PF.tile([128, TQ], F32, tag="th", bufs=2)
                        nc.vector.tensor_tensor(th[:], ps[:], r2_bc[:], ALU.mult)
                        nc.scalar.activation(h[:, m, :], th[:], AF.Gelu,
                                             bias=b1_t[:, m:m + 1])
                for m in range(DT):
                    if m == 0:
                        w2c = w2c0
                    else:
                        w2c = PF.tile([128, FT // 2, 256], BF16, tag="w2c",
                                      bufs=3)
                        nc.sync.dma_start(w2c[:], w2.ap()[m])
                    ps = PFP.tile([128, TQ], F32, tag="ops")
                    for k in range(FT):
                        nc.tensor.matmul(
                            ps[:], w2c[:, k // 2, (k % 2) * 128:(k % 2) * 128 + 128],
                            h[:, k, :],
                            start=(k == 0), stop=(k == FT - 1))
                    t2 = PF.tile([128, TQ], F32, tag="t2", bufs=2)
                    nc.vector.scalar_tensor_tensor(t2[:], ps[:], b2_t[:, m:m + 1],
                                                   x2T[:, m, :], ALU.add, ALU.add)
                    ob = PF.tile([128, TQ], F32, tag="ob", bufs=2)
                    nc.vector.tensor_tensor(ob[:], t2[:], mu2_bc[:], ALU.add)
                    nc.sync.dma_start(
                        outT.ap().rearrange("(a p) t -> p a t", p=128)[:, m, :],
                        ob[:])
    nc.compile()
    return nc


_CACHE = {}


def _get_nc(c: CFG):
    key = (c.D, c.TB, c.TQ, c.NH, c.HFF)
    if key not in _CACHE:
        _CACHE[key] = build_nc(c)
    return _CACHE[key]


def core_perm(c: CFG, j: int):
    """Token-tile permutation for core j: within each group of 4 tiles the
    core's tile (index j in the group) moves to the group end."""
    tiles = []
    for a in range(c.TB // 512):
        grp = [4 * a + b for b in range(4) if b != j] + [4 * a + j]
        tiles.extend(grp)
    return tiles


def make_in_maps(c: CFG, x, mask, ln1_g, ln1_b, qkv_w, qkv_b, proj_w, proj_b,
                 ln2_g, ln2_b, w1, b1, w2, b2):
    D, TB, TQ, DT, FT = c.D, c.TB, c.TQ, c.DT, c.FT
    B = x.shape[0]
    ncg = TB // TQ  # query groups per batch

    f = np.float32
    bf = ml_dtypes.bfloat16
    g1 = ln1_g.astype(f)
    sc = 1.0 / np.sqrt(c.HD)
    DT, FT, VN, NVB = c.DT, c.FT, c.VN, c.NVB

    def tile_kp(w):  # [D, M] -> [128, DT, M] (partition-contiguous slabs)
        return np.ascontiguousarray(w.reshape(DT, 128, -1).transpose(1, 0, 2))

    wq_f = tile_kp((qkv_w[:, :D] * g1[:, None] * sc).astype(bf))
    wk_f = tile_kp((qkv_w[:, D:2 * D] * g1[:, None]).astype(bf))
    wv_b = (qkv_w[:, 2 * D:] * g1[:, None]).astype(bf)
    wv_f = np.ascontiguousarray(
        wv_b.reshape(DT, 128, NVB, VN).transpose(2, 1, 0, 3))
    bq_f = ((qkv_b[:D] + ln1_b @ qkv_w[:, :D]) * sc).astype(f)
    bk_f = (qkv_b[D:2 * D] + ln1_b @ qkv_w[:, D:2 * D]).astype(f)
    bv_f = (qkv_b[2 * D:] + ln1_b @ qkv_w[:, 2 * D:]).astype(f)
    w1g = (w1 * ln2_g.astype(f)[:, None]).astype(bf)
    w1_f = np.ascontiguousarray(
        w1g.reshape(DT, 128, FT // 2, 256).transpose(2, 1, 0, 3))
    b1_f = (b1 + ln2_b @ w1).astype(f)
    pw_f = tile_kp(np.asarray(proj_w, f).astype(bf))
    w2_f = np.ascontiguousarray(
        np.asarray(w2, f).astype(bf).reshape(FT, 128, DT, 128)
        .transpose(2, 1, 0, 3).reshape(DT, 128, FT // 2, 256))

    def btile(v, nt):
        return np.ascontiguousarray(v.reshape(nt, 128).T, f)

    z01 = np.asarray(mask[0, 0], f)  # [T,T] rows=queries, cols=keys

    shared = {
        "wq": wq_f, "wk": wk_f, "wv": wv_f, "pw": pw_f,
        "w1": w1_f, "w2": w2_f,
        "bq": btile(bq_f, DT), "bk": btile(bk_f, DT),
        "bv": np.ascontiguousarray(bv_f.reshape(1, D).astype(bf)),
        "pb": btile(proj_b.astype(f), DT),
        "b1": btile(b1_f, FT), "b2": btile(b2.astype(f), DT),
    }
    in_maps = []
    for core in range(B * ncg):
        b, j = core // ncg, core % ncg
        perm = core_perm(c, j)
        ptok = np.concatenate([np.arange(t * 128, (t + 1) * 128)
                               for t in perm])
        qtok = np.concatenate([np.arange((4 * i + j) * 128,
                                         (4 * i + j + 1) * 128)
                               for i in range(TQ // 128)])
        m = dict(shared)
        m["xT"] = np.ascontiguousarray(x[b][ptok].T, f)            # [D, TB]
        m["xqT"] = np.ascontiguousarray(x[b][qtok].T, f)           # [D, TQ]
        mm = np.empty((128, c.NTT, 128), np.float32)
        for kt in range(c.NTT):
            a = kt // 4
            gk, gq = perm[kt], 4 * a + j
            mm[:, kt, :] = z01[gq * 128:(gq + 1) * 128,
                               gk * 128:(gk + 1) * 128].T
        m["maskM"] = mm.astype(bf)
        in_maps.append(m)
    return in_maps


def assemble_out(c: CFG, results, B):
    ncg = c.TB // c.TQ
    out = np.empty((B, c.TB, c.D), np.float32)
    for core, res in enumerate(results):
        b, j = core // ncg, core % ncg
        o = res["outT"].T                                   # [TQ, D]
        for i in range(c.TQ // 128):
            t = 4 * i + j
            out[b, t * 128:(t + 1) * 128, :] = o[i * 128:(i + 1) * 128, :]
    return out


def kernel(x, mask, ln1_g, ln1_b, qkv_w, qkv_b, proj_w, proj_b,
           ln2_g, ln2_b, w1, b1, w2, b2):
    x = np.asarray(x, np.float32)
    c = CFG(D=x.shape[2], TB=x.shape[1], TQ=x.shape[1] // 4,
            NH=16, HD=64, HFF=4 * x.shape[2])
    nc = _get_nc(c)
    in_maps = make_in_maps(c, x, np.asarray(mask), *[np.asarray(a, np.float32)
                           for a in (ln1_g, ln1_b, qkv_w, qkv_b, proj_w, proj_b,
                                     ln2_g, ln2_b, w1, b1, w2, b2)])
    res = run_bass_kernel_spmd(nc, in_maps, core_ids=list(range(len(in_maps))))
    return assemble_out(c, res.results, x.shape[0])


if __name__ == "__main__":
    c = CFG()
    nc = build_nc(c)
    print("built ok")


# revision 75
# speedup vs baseline: 1.2737x; 1.0018x over previous
"""Trainium2 Bass kernel for a dense transformer block (pre-LN, causal attention, GELU FFN).

Sharding: 8 cores = 2 batches x 4 query-groups of 512 tokens, communication
free. Per-core the batch's token tiles are PERMUTED on host so that this
core's 4 query tiles sit at static positions {3,7,11,15} (within each group
of 4 tiles, the core's tile is moved to the group end, others stay in
ascending order). Keys stay causal-compatible: a query at position 4a+3 only
needs key positions 0..4a+3, so scores/AV run with causally truncated width
and only one diagonal 128x128 sub-tile per key tile needs mask application.
Q reuses the X-phase centered activations and row stats (no second LN pass).
All activations are feature-major [d, tokens]; LayerNorm is folded into
host-prepared weights plus per-token column stats applied at PSUM evacuation.
"""

import sys

sys.path.insert(0, "/opt/trn_rl_repo")

import numpy as np
import ml_dtypes

import concourse.bass as bass
import concourse.tile as tile
from concourse import bacc, mybir
from concourse.bass import ts
from concourse.bass_utils import run_bass_kernel_spmd

F32 = mybir.dt.float32
F32R = mybir.dt.float32r
BF16 = mybir.dt.bfloat16
AF = mybir.ActivationFunctionType
ALU = mybir.AluOpType

EPS = 1e-5


class CFG:
    def __init__(self, D=1024, TB=2048, TQ=512, NH=16, HD=64, HFF=4096):
        self.D, self.TB, self.TQ, self.NH, self.HD, self.HFF = D, TB, TQ, NH, HD, HFF
        self.DT = D // 128          # d_model tiles
        self.FT = HFF // 128        # ffn tiles
        self.NTT = TB // 128        # key token tiles
        self.NBLK = TB // 512       # 512-token kv blocks
        self.VN = min(512, D)       # V matmul free width
        self.NVB = D // self.VN     # V col blocks
        self.HPV = self.VN // HD    # heads per V col block
        assert NH == 2 * self.DT and HD == 64


def build_nc(c: CFG):
    nc = bacc.Bacc()
    D, TB, TQ, DT, FT, NTT, NBLK = c.D, c.TB, c.TQ, c.DT, c.FT, c.NTT, c.NBLK
    XW = 256                      # x-stream block width
    NXB = TB // XW
    NQT = TQ // 128               # query tiles per core

    xT = nc.dram_tensor("xT", [D, TB], F32R, kind="ExternalInput")
    xqT = nc.dram_tensor("xqT", [D, TQ], F32R, kind="ExternalInput")
    maskM = nc.dram_tensor("maskM", [128, NTT, 128], BF16, kind="ExternalInput")
    wq = nc.dram_tensor("wq", [128, DT, D], BF16, kind="ExternalInput")
    wk = nc.dram_tensor("wk", [128, DT, D], BF16, kind="ExternalInput")
    wv = nc.dram_tensor("wv", [c.NVB, 128, DT, c.VN], BF16, kind="ExternalInput")
    pw = nc.dram_tensor("pw", [128, DT, D], BF16, kind="ExternalInput")
    w1 = nc.dram_tensor("w1", [FT // 2, 128, DT, 256], BF16,
                        kind="ExternalInput")
    w2 = nc.dram_tensor("w2", [DT, 128, FT // 2, 256], BF16,
                        kind="ExternalInput")
    bq = nc.dram_tensor("bq", [128, DT], F32, kind="ExternalInput")
    bk = nc.dram_tensor("bk", [128, DT], F32, kind="ExternalInput")
    bv = nc.dram_tensor("bv", [1, D], BF16, kind="ExternalInput")
    pb = nc.dram_tensor("pb", [128, DT], F32, kind="ExternalInput")
    b1 = nc.dram_tensor("b1", [128, FT], F32, kind="ExternalInput")
    b2 = nc.dram_tensor("b2", [128, DT], F32, kind="ExternalInput")
    outT = nc.dram_tensor("outT", [D, TQ], F32, kind="ExternalOutput")

    def dram3(t):  # [ (a p), m ] -> [p, a, m]
        return t.ap().rearrange("(a p) m -> p a m", p=128)

    with tile.TileContext(nc) as tc:
        with tc.tile_pool(name="persist", bufs=1) as P:
            x2T = P.tile([128, DT, TQ], BF16)
            bvb = P.tile([128, D], BF16)
            r_row = P.tile([1, TB], F32)
            rt = P.tile([128, NTT], F32)
            ones = P.tile([128, 1], F32R)
            eps_t = P.tile([1, 1], F32)
            id11 = P.tile([1, 1], F32)
            mu2_r = P.tile([1, TQ], F32)
            r2_r = P.tile([1, TQ], F32)
            bq_t = P.tile([128, DT], F32)
            bk_t = P.tile([128, DT], F32)
            pb_t = P.tile([128, DT], F32)
            b2_t = P.tile([128, DT], F32)
            b1_t = P.tile([128, FT], F32)

            ones_f = P.tile([128, 1], F32)
            ones_b = P.tile([128, 1], BF16)
            nc.vector.memset(ones_f[:], 1.0)
            nc.vector.tensor_copy(ones[:], ones_f[:])
            nc.vector.tensor_copy(ones_b[:], ones_f[:])
            nc.vector.memset(eps_t[:], EPS)
            nc.vector.memset(id11[:], 1.0)
            warm = P.tile([1, 1], F32)
            nc.scalar.dma_start(bq_t[:], bq.ap())
            nc.scalar.dma_start(bk_t[:], bk.ap())
            nc.scalar.dma_start(pb_t[:], pb.ap())
            nc.scalar.dma_start(b2_t[:], b2.ap())
            nc.scalar.dma_start(b1_t[:], b1.ap())

            # stats over the feature (partition) axis + centering, feature-major
            def stats_center(xsrc, W, r_slice, mu_bc, r_bc, dst, sqp, stp):
                sum_ps = stp.tile([1, W], F32, tag="sum_ps")
                sq_ps = stp.tile([1, W], F32, tag="sq_ps")
                for k in range(DT):
                    nc.tensor.matmul(sum_ps[:], ones_b[:], xsrc[:, k, :],
                                     start=(k == 0), stop=(k == DT - 1))
                for k in range(DT):
                    sq = sqp.tile([128, W], BF16, tag="sq")
                    nc.scalar.square(sq[:], xsrc[:, k, :])
                    nc.tensor.matmul(sq_ps[:], ones_b[:], sq[:],
                                     start=(k == 0), stop=(k == DT - 1))
                mu_r = sqp.tile([1, W], F32, tag="mu_r", bufs=1)
                nc.vector.tensor_scalar(mu_r[:], sum_ps[:], 1.0 / D, None, ALU.mult)
                musq = sqp.tile([1, W], F32, tag="musq", bufs=1)
                nc.vector.tensor_tensor(musq[:], mu_r[:], mu_r[:], ALU.mult)
                var = sqp.tile([1, W], F32, tag="var", bufs=1)
                nc.vector.scalar_tensor_tensor(var[:], sq_ps[:], 1.0 / D, musq[:],
                                               ALU.mult, ALU.subtract)
                std = sqp.tile([1, W], F32, tag="std", bufs=1)
                nc.scalar.activation(std[:], var[:], AF.Sqrt, bias=eps_t[:])
                nc.vector.reciprocal(r_slice, std[:])
                nc.gpsimd.partition_broadcast(mu_bc[:], mu_r[:])
                if r_bc is not None:
                    nc.gpsimd.partition_broadcast(r_bc[:], r_slice)
                # split centering DVE/Pool to halve the serial chain
                for k in range(DT):
                    eng = nc.vector if k % 3 else nc.gpsimd
                    eng.tensor_tensor(dst[:, k, :], xsrc[:, k, :], mu_bc[:],
                                      ALU.subtract)

            with tc.tile_pool(name="kvres", bufs=1) as KV:
                kT = KV.tile([128, DT, TB], BF16)
                V = KV.tile([128, NTT, c.NH * 65], BF16)
                qT = KV.tile([128, DT, TQ], BF16)
                xt = KV.tile([128, DT, TB], BF16)
                vone = V[:].rearrange("p t (h c) -> p t h c", c=65)

                # ----- Phase X+K fused: stream/center x, K per ready 512-block -----
                with tc.tile_pool(name="phx", bufs=1) as XP:
                    wq_t = XP.tile([128, DT, D], BF16)
                    with tc.tile_pool(name="phxs", bufs=2) as PX, \
                         tc.tile_pool(name="phxs_ps", bufs=2, space="PSUM") as PXP:
                        wk_t = PX.tile([128, DT, D], BF16, bufs=1)

                        # software pipeline: block i does PE sums + Act squares;
                        # block i-1 does sq-matmuls + finalize + centering, so
                        # PE never waits on the Act/DVE stats chain.
                        def finish(pv):
                            xi, off, xb, sum_ps, sq = pv
                            sq_ps = PXP.tile([1, XW], F32, tag="sq_ps")
                            for k in range(DT):
                                nc.tensor.matmul(sq_ps[:], ones_b[:],
                                                 sq[:, k, :],
                                                 start=(k == 0),
                                                 stop=(k == DT - 1))
                            mu_r = PX.tile([1, XW], F32, tag="mu_r")
                            nc.vector.tensor_scalar(mu_r[:], sum_ps[:], 1.0 / D,
                                                    None, ALU.mult)
                            musq = PX.tile([1, XW], F32, tag="musq")
                            nc.vector.tensor_tensor(musq[:], mu_r[:], mu_r[:],
                                                    ALU.mult)
                            var = PX.tile([1, XW], F32, tag="var")
                            nc.vector.scalar_tensor_tensor(
                                var[:], sq_ps[:], 1.0 / D, musq[:],
                                ALU.mult, ALU.subtract)
                            std = PX.tile([1, XW], F32, tag="std")
                            nc.scalar.activation(std[:], var[:], AF.Sqrt,
                                                 bias=eps_t[:])
                            nc.vector.reciprocal(r_row[0:1, off:off + XW],
                                                 std[:])
                            mu_bc = PX.tile([128, XW], F32, tag="mu_bc")
                            nc.gpsimd.partition_broadcast(mu_bc[:], mu_r[:])
                            for k in range(DT):
                                nc.vector.tensor_tensor(
                                    xt[:, k, off:off + XW], xb[:, k, :],
                                    mu_bc[:], ALU.subtract)

                        def do_k(b4):
                            off4 = b4 * 512
                            rb4 = PX.tile([128, 512], F32, tag="rb4", bufs=1)
                            nc.gpsimd.partition_broadcast(
                                rb4[:], r_row[0:1, off4:off4 + 512])
                            for m in range(DT):
                                ps = PXP.tile([128, 512], F32, tag="kps")
                                for k in range(DT):
                                    nc.tensor.matmul(
                                        ps[:], wk_t[:, k, ts(m, 128)],
                                        xt[:, k, off4:off4 + 512],
                                        start=(k == 0), stop=(k == DT - 1))
                                ev = PX.tile([128, 512], F32, tag="kev",
                                             bufs=2)
                                nc.vector.tensor_tensor(ev[:], ps[:], rb4[:],
                                                        ALU.mult)
                                nc.scalar.activation(
                                    kT[:, m, off4:off4 + 512], ev[:],
                                    AF.Identity, bias=bk_t[:, m:m + 1])
                            # rt transposes after K: stats chain long done
                            for tt in range(4):
                                g = b4 * 4 + tt
                                rt_ps = PXP.tile([128, 1], F32, tag="rt_ps")
                                nc.tensor.transpose(
                                    rt_ps[:],
                                    r_row[0:1, g * 128:(g + 1) * 128],
                                    id11[:])
                                nc.vector.tensor_copy(rt[:, g:g + 1],
                                                      rt_ps[:])

                        prev = None
                        for xi in range(NXB):
                            off = xi * XW
                            xb = PX.tile([128, DT, XW], F32R, tag="xb", bufs=2)
                            nc.sync.dma_start(xb[:],
                                              dram3(xT)[:, :, off:off + XW])
                            if xi == 1:
                                for kk in range(4):
                                    nc.sync.dma_start(
                                        wk_t[:, 2 * kk:2 * kk + 2, :],
                                        wk.ap()[:, 2 * kk:2 * kk + 2, :])
                            if xi == 2:
                                nc.sync.dma_start(wq_t[:], wq.ap())
                            sum_ps = PXP.tile([1, XW], F32, tag="sum_ps")
                            for k in range(DT):
                                nc.tensor.matmul(sum_ps[:], ones[:], xb[:, k, :],
                                                 start=(k == 0),
                                                 stop=(k == DT - 1))
                            sq = PX.tile([128, DT, XW], BF16, tag="sq", bufs=2)
                            nc.scalar.square(sq[:, 0:DT // 2, :],
                                             xb[:, 0:DT // 2, :])
                            nc.scalar.square(sq[:, DT // 2:DT, :],
                                             xb[:, DT // 2:DT, :])
                            if prev is not None:
                                finish(prev)
                                if xi >= 2 and xi % 2 == 0:
                                    do_k(xi // 2 - 1)
                            prev = (xi, off, xb, sum_ps, sq)
                        finish(prev)
                        do_k(NBLK - 1)

                    nc.vector.memset(vone[:, :, :, 64:65], 1.0)
                    bv_r = XP.tile([1, D], BF16)
                    nc.scalar.dma_start(bv_r[:], bv.ap())
                    nc.gpsimd.partition_broadcast(bvb[:], bv_r[:])

                    # ---------------- Phase Q ----------------
                    # q tiles sit at static positions {3,7,11,15}: reuse the
                    # centered xt and r_row stats from the X phase.
                    with tc.tile_pool(name="phq", bufs=2) as PQ, \
                         tc.tile_pool(name="phq_ps", bufs=2, space="PSUM") as PQP:
                        nc.scalar.activation(warm[:], eps_t[:], AF.Exp)
                        nc.scalar.activation(warm[:], eps_t[:], AF.Gelu,
                                             bias=eps_t[:])
                        rq_r = PQ.tile([1, TQ], F32, bufs=1)
                        nc.vector.tensor_copy(
                            rq_r[:],
                            r_row[0:1, :].rearrange("o (g w) -> o g w", w=512)
                            [:, :, 384:512])
                        rq_bc = PQ.tile([128, TQ], F32, bufs=1)
                        nc.gpsimd.partition_broadcast(rq_bc[:], rq_r[:])
                        xt_q = xt[:].rearrange("p d (g w) -> p d g w", w=512)[
                            :, :, :, 384:512]
                        for m in range(DT):
                            ps = PQP.tile([128, TQ], F32, tag="qps")
                            for k in range(DT):
                                nc.tensor.matmul(
                                    ps[:].rearrange("p (g w) -> p g w", w=128),
                                    wq_t[:, k, ts(m, 128)], xt_q[:, k],
                                    start=(k == 0), stop=(k == DT - 1))
                            ev = PQ.tile([128, TQ], F32, tag="qev", bufs=2)
                            nc.vector.tensor_tensor(ev[:], ps[:], rq_bc[:],
                                                    ALU.mult)
                            nc.scalar.activation(qT[:, m, :], ev[:],
                                                 AF.Identity,
                                                 bias=bq_t[:, m:m + 1])

                # ------------- Phase attention (V folded into hp 0) -------------
                # php wraps pha so proj's weights/residual stream in during
                # attention (distinct SBUF -> no WAR serialization).
                with tc.tile_pool(name="php", bufs=1) as PP:
                  pw_t = PP.tile([128, DT, D], BF16)
                  xq2 = PP.tile([128, DT, TQ], F32R)
                  # per-k tiles so proj passes only dep on the heads they read
                  aoT = [PP.tile([128, TQ], BF16, name=f"aoT{k}")
                         for k in range(DT)]
                  with tc.tile_pool(name="pha", bufs=2) as PA:
                    mk = PA.tile([128, NTT, 128], BF16, bufs=1)
                    nc.scalar.dma_start(mk[:], maskM.ap())

                    def do_v(g):
                        for n in range(c.NVB):
                            ps = PPPP.tile([128, c.VN], F32, tag="ppool")
                            for k in range(DT):
                                nc.tensor.matmul(ps[:], xt[:, k, ts(g, 128)],
                                                 wvh[:, n, k, :],
                                                 start=(k == 0),
                                                 stop=(k == DT - 1))
                            dst = vone[:, g, n * c.HPV:(n + 1) * c.HPV, 0:64]
                            nc.vector.scalar_tensor_tensor(
                                dst, ps[:].rearrange("p (h c) -> p h c", c=64),
                                rt[:, g:g + 1],
                                bvb[:, ts(n, c.VN)].rearrange(
                                    "p (h c) -> p h c", c=64),
                                ALU.mult, ALU.add)

                    def proj_ks(ms, ks, first):
                        # proj k-tile chain for m in ms, accumulated into part
                        for m in ms:
                            ps = PPPP.tile([128, TQ], F32, tag="ppool",
                                           name=f"pkp{ks[0]}_{m}")
                            for k in ks:
                                nc.tensor.matmul(ps[:], pw_t[:, k, ts(m, 128)],
                                                 aoT[k][:], start=(k == ks[0]),
                                                 stop=(k == ks[-1]))
                            if first:
                                nc.vector.scalar_tensor_tensor(
                                    part[:, m, :], ps[:], 1.0, xq2[:, m, :],
                                    ALU.mult, ALU.add)
                            else:
                                nc.vector.tensor_tensor(part[:, m, :],
                                                        part[:, m, :], ps[:],
                                                        ALU.add)

                    def do_hp(hp):
                        # [65, TQ] f32 = one 2KB PSUM bank per half: one
                        # accumulation group each; truncated-width matmuls
                        # accumulate sub-columns, single stop at the last
                        # key tile closes the bank.
                        av0 = PAVP.tile([65, TQ], F32, tag="av0", bufs=1)
                        av1 = PAVP.tile([65, TQ], F32, tag="av1", bufs=1)
                        h0, h1 = 2 * hp, 2 * hp + 1
                        pq = []

                        def do_av(tk, p0, p1):
                            qlo = (tk // 4) * 128
                            for hh, av, pp in ((h0, av0, p0), (h1, av1, p1)):
                                nc.tensor.matmul(
                                    av[:, qlo:TQ],
                                    V[:, tk, hh * 65:(hh + 1) * 65],
                                    pp, start=(tk == 0),
                                    stop=(tk == NTT - 1))

                        # same-width key tiles grouped into one PSUM tile so
                        # exp and mask batch into single instructions
                        groups = [[0], [1], [2], [3], [4], [5], [6], [7],
                                  [8, 9], [10, 11], [12, 13, 14, 15]]
                        for grp in groups:
                            if hp == 0:
                                for tk in grp:
                                    do_v(tk)
                            ng = len(grp)
                            qlo = (grp[0] // 4) * 128
                            w = TQ - qlo
                            s01 = PAP.tile([128, 2, TQ], F32, tag="s01",
                                           bufs=2)
                            p01 = PA.tile([128, 2, TQ], BF16, tag="p01",
                                          bufs=4)
                            if ng == 1:
                                sv = s01[:].unsqueeze(2)[:, :, :, 0:w]
                                pv = p01[:].unsqueeze(2)[:, :, :, 0:w]
                            else:
                                sv = s01[:].rearrange("p h (g q) -> p h g q",
                                                      q=w)
                                pv = p01[:].rearrange("p h (g q) -> p h g q",
                                                      q=w)
                            for gi, tk in enumerate(grp):
                                nc.tensor.matmul(sv[:, 0, gi, :],
                                                 kT[0:64, hp, ts(tk, 128)],
                                                 qT[0:64, hp, qlo:TQ],
                                                 start=True, stop=True)
                                nc.tensor.matmul(sv[:, 1, gi, :],
                                                 kT[64:128, hp, ts(tk, 128)],
                                                 qT[64:128, hp, qlo:TQ],
                                                 start=True, stop=True)
                            nc.scalar.activation(p01[:, :, 0:ng * w],
                                                 s01[:, :, 0:ng * w], AF.Exp)
                            mkb = mk[:, grp[0]:grp[0] + ng, :].unsqueeze(
                                1).to_broadcast((128, 2, ng, 128))
                            nc.vector.tensor_tensor(pv[:, :, :, 0:128],
                                                    pv[:, :, :, 0:128], mkb,
                                                    ALU.mult)
                            for gi, tk in enumerate(grp):
                                pq.append((tk, pv[:, 0, gi, :],
                                           pv[:, 1, gi, :]))
                                if len(pq) > 2:
                                    do_av(*pq.pop(0))
                        while pq:
                            do_av(*pq.pop(0))
                        recs, rbcs = [], []
                        for av, half in ((av0, 0), (av1, 1)):
                            rec = PA.tile([1, TQ], F32, tag="rec", bufs=2)
                            nc.vector.reciprocal(rec[:], av[64:65, :])
                            recs.append(rec)
                        for half in (0, 1):
                            rbc = PA.tile([64, TQ], F32, tag="rbc", bufs=2)
                            nc.gpsimd.partition_broadcast(rbc[:], recs[half][:],
                                                          channels=64)
                            rbcs.append(rbc)
                        for av, half in ((av0, 0), (av1, 1)):
                            nc.vector.tensor_tensor(
                                aoT[hp][64 * half:64 * (half + 1), :],
                                av[0:64, :], rbcs[half][:], ALU.mult)

                    with tc.tile_pool(name="pha_ps", bufs=2, space="PSUM") as PAP, \
                         tc.tile_pool(name="pav_ps", bufs=1, space="PSUM") as PAVP, \
                         tc.tile_pool(name="ppp_ps", bufs=2, space="PSUM") as PPPP:
                        with tc.tile_pool(name="phv", bufs=1) as PV:
                            wvh = PV.tile([128, c.NVB, DT, c.VN], BF16)
                            for n in range(c.NVB):
                                nc.sync.dma_start(wvh[:, n], wv.ap()[n])
                            nc.sync.dma_start(pw_t[:], pw.ap())
                            nc.scalar.dma_start(xq2[:], dram3(xqT))
                            do_hp(0)
                        with tc.tile_pool(name="phpart", bufs=1) as PT:
                            part = PT.tile([128, DT, TQ], F32)
                            # proj filler at hp h's end over finished aoT
                            for hp in range(1, c.NH // 2):
                                do_hp(hp)
                                if hp >= 2:
                                    ms = (range(0, 4) if hp % 2 == 0
                                          else range(4, DT))
                                    k0 = 2 * ((hp - 2) // 2)
                                    proj_ks(ms, [k0, k0 + 1], hp < 4)

                            # ------------- proj tail: k tiles 6,7 -------------
                            # hoist k6 chains for m0/m1 ahead of the k7s
                            # (which wait on hp7's evac) to keep PE fed
                            def p2_chain(ps, m, ks):
                                for k in ks:
                                    nc.tensor.matmul(
                                        ps[:], pw_t[:, k, ts(m, 128)],
                                        aoT[k][:],
                                        start=(k == DT - 2),
                                        stop=(k == DT - 1))

                            pss = {}
                            for m in (0, 1):
                                pss[m] = PPPP.tile([128, TQ], F32,
                                                   tag="ppool",
                                                   name=f"p2ps{m}")
                                p2_chain(pss[m], m, [DT - 2])
                            sqs = []
                            for m in range(DT):
                                if m in pss:
                                    ps = pss[m]
                                    p2_chain(ps, m, [DT - 1])
                                else:
                                    ps = PPPP.tile([128, TQ], F32,
                                                   tag="ppool",
                                                   name=f"p2ps{m}")
                                    p2_chain(ps, m, [DT - 2, DT - 1])
                                nc.vector.scalar_tensor_tensor(
                                    x2T[:, m, :], ps[:], pb_t[:, m:m + 1],
                                    part[:, m, :], ALU.add, ALU.add)
                                # pre-square for the FFN ln2 stats while the
                                # Act engine is otherwise idle
                                sq2 = PA.tile([128, 2, TQ], BF16, tag="p01",
                                              bufs=4, name=f"sq2_{m}")
                                nc.scalar.square(sq2[:, 0, :], x2T[:, m, :])
                                sqs.append(sq2)
                            # ln2 stats matmuls + scalar chain, still inside
                            # the attention scope (overlaps the proj tail)
                            sum2 = PAVP.tile([1, TQ], F32, tag="av0",
                                             name="sum2")
                            sq2p = PAVP.tile([1, TQ], F32, tag="av1",
                                             name="sq2p")
                            for k in range(DT):
                                nc.tensor.matmul(sum2[:], ones_b[:],
                                                 x2T[:, k, :],
                                                 start=(k == 0),
                                                 stop=(k == DT - 1))
                            for k in range(DT):
                                nc.tensor.matmul(sq2p[:], ones_b[:],
                                                 sqs[k][:, 0, :],
                                                 start=(k == 0),
                                                 stop=(k == DT - 1))
                            nc.vector.tensor_scalar(mu2_r[:], sum2[:],
                                                    1.0 / D, None, ALU.mult)
                            musq2 = PA.tile([1, TQ], F32, tag="rec",
                                            name="musq2")
                            nc.vector.tensor_tensor(musq2[:], mu2_r[:],
                                                    mu2_r[:], ALU.mult)
                            var2 = PA.tile([1, TQ], F32, tag="rec",
                                           name="var2")
                            nc.vector.scalar_tensor_tensor(
                                var2[:], sq2p[:], 1.0 / D, musq2[:],
                                ALU.mult, ALU.subtract)
                            std2 = PA.tile([1, TQ], F32, tag="rec",
                                           name="std2")
                            nc.scalar.activation(std2[:], var2[:], AF.Sqrt,
                                                 bias=eps_t[:])
                            nc.vector.reciprocal(r2_r[:], std2[:])

            # ---------------- Phase FFN ----------------
            with tc.tile_pool(name="phf", bufs=2) as PF, \
                 tc.tile_pool(name="phf_ps", bufs=2, space="PSUM") as PFP:
                h = PF.tile([128, FT, TQ], BF16, bufs=1)
                xc = PF.tile([128, DT, TQ], BF16, bufs=1)
                mu2_bc = PF.tile([128, TQ], F32, bufs=1)
                r2_bc = PF.tile([128, TQ], F32, bufs=1)
                w1c0 = PF.tile([128, DT, 256], BF16, bufs=1)
                nc.sync.dma_start(w1c0[:], w1.ap()[0])
                w2c0 = PF.tile([128, FT // 2, 256], BF16, tag="w2c", bufs=3)
                nc.scalar.dma_start(w2c0[:], w2.ap()[0])
                # stats were computed during the proj tail; broadcast, then
                # center per-k with the first FFN1 chains interleaved so PE
                # chases the centering instead of waiting for all of it
                nc.gpsimd.partition_broadcast(mu2_bc[:], mu2_r[:])
                nc.gpsimd.partition_broadcast(r2_bc[:], r2_r[:])
                ps_i0 = PFP.tile([128, TQ], F32, tag="hps")
                ps_i1 = PFP.tile([128, TQ], F32, tag="hps")
                for k in range(DT):
                    eng = nc.gpsimd if k % 3 == 2 else nc.vector
                    eng.tensor_tensor(xc[:, k, :], x2T[:, k, :], mu2_bc[:],
                                      ALU.subtract)
                    nc.tensor.matmul(ps_i0[:], w1c0[:, k, ts(0, 128)],
                                     xc[:, k, :],
                                     start=(k == 0), stop=(k == DT - 1))
                    nc.tensor.matmul(ps_i1[:], w1c0[:, k, ts(1, 128)],
                                     xc[:, k, :],
                                     start=(k == 0), stop=(k == DT - 1))
                for m in (0, 1):
                    ps = ps_i0 if m == 0 else ps_i1
                    th = PF.tile([128, TQ], F32, tag="th", bufs=2,
                                 name=f"th_i{m}")
                    nc.vector.tensor_tensor(th[:], ps[:], r2_bc[:], ALU.mult)
                    nc.scalar.activation(h[:, m, :], th[:], AF.Gelu,
                                         bias=b1_t[:, m:m + 1])
                for mg in range(1, FT // 2):
                    w1c = PF.tile([128, DT, 256], BF16, tag="w1c", bufs=4)
                    nc.sync.dma_start(w1c[:], w1.ap()[mg])
                    for mi in range(2):
                        m = 2 * mg + mi
                        ps = PFP.tile([128, TQ], F32, tag="hps")
                        for k in range(DT):
                            nc.tensor.matmul(ps[:], w1c[:, k, ts(mi, 128)],
                                             xc[:, k, :],
                                             start=(k == 0), stop=(k == DT - 1))
                        th = PF.tile([128, TQ], F32, tag="th", bufs=2)
                        nc.vector.tensor_tensor(th[:], ps[:], r2_bc[:], ALU.mult)
                        nc.scalar.activation(h[:, m, :], th[:], AF.Gelu,
                                             bias=b1_t[:, m:m + 1])
                for m in range(DT):
                    if m == 0:
                        w2c = w2c0
                    else:
                        w2c = PF.tile([128, FT // 2, 256], BF16, tag="w2c",
                                      bufs=3)
                        nc.sync.dma_start(w2c[:], w2.ap()[m])
                    ps = PFP.tile([128, TQ], F32, tag="ops")
                    for k in range(FT):
                        nc.tensor.matmul(
                            ps[:], w2c[:, k // 2, (k % 2) * 128:(k % 2) * 128 + 128],
                            h[:, k, :],
                            start=(k == 0), stop=(k == FT - 1))
                    ob = PF.tile([128, TQ], F32, tag="ob", bufs=2)
                    nc.vector.scalar_tensor_tensor(ob[:], ps[:], b2_t[:, m:m + 1],
                                                   x2T[:, m, :], ALU.add, ALU.add)
                    nc.sync.dma_start(
                        outT.ap().rearrange("(a p) t -> p a t", p=128)[:, m, :],
                        ob[:])
    nc.compile()
    return nc


_CACHE = {}


def _get_nc(c: CFG):
    key = (c.D, c.TB, c.TQ, c.NH, c.HFF)
    if key not in _CACHE:
        _CACHE[key] = build_nc(c)
    return _CACHE[key]


def core_perm(c: CFG, j: int):
    """Token-tile permutation for core j: within each group of 4 tiles the
    core's tile (index j in the group) moves to the group end."""
    tiles = []
    for a in range(c.TB // 512):
        grp = [4 * a + b for b in range(4) if b != j] + [4 * a + j]
        tiles.extend(grp)
    return tiles


def make_in_maps(c: CFG, x, mask, ln1_g, ln1_b, qkv_w, qkv_b, proj_w, proj_b,
                 ln2_g, ln2_b, w1, b1, w2, b2):
    D, TB, TQ, DT, FT = c.D, c.TB, c.TQ, c.DT, c.FT
    B = x.shape[0]
    ncg = TB // TQ  # query groups per batch

    f = np.float32
    bf = ml_dtypes.bfloat16
    g1 = ln1_g.astype(f)
    sc = 1.0 / np.sqrt(c.HD)
    DT, FT, VN, NVB = c.DT, c.FT, c.VN, c.NVB

    def tile_kp(w):  # [D, M] -> [128, DT, M] (partition-contiguous slabs)
        return np.ascontiguousarray(w.reshape(DT, 128, -1).transpose(1, 0, 2))

    wq_f = tile_kp((qkv_w[:, :D] * g1[:, None] * sc).astype(bf))
    wk_f = tile_kp((qkv_w[:, D:2 * D] * g1[:, None]).astype(bf))
    wv_b = (qkv_w[:, 2 * D:] * g1[:, None]).astype(bf)
    wv_f = np.ascontiguousarray(
        wv_b.reshape(DT, 128, NVB, VN).transpose(2, 1, 0, 3))
    bq_f = ((qkv_b[:D] + ln1_b @ qkv_w[:, :D]) * sc).astype(f)
    bk_f = (qkv_b[D:2 * D] + ln1_b @ qkv_w[:, D:2 * D]).astype(f)
    bv_f = (qkv_b[2 * D:] + ln1_b @ qkv_w[:, 2 * D:]).astype(f)
    w1g = (w1 * ln2_g.astype(f)[:, None]).astype(bf)
    w1_f = np.ascontiguousarray(
        w1g.reshape(DT, 128, FT // 2, 256).transpose(2, 1, 0, 3))
    b1_f = (b1 + ln2_b @ w1).astype(f)
    pw_f = tile_kp(np.asarray(proj_w, f).astype(bf))
    w2_f = np.ascontiguousarray(
        np.asarray(w2, f).astype(bf).reshape(FT, 128, DT, 128)
        .transpose(2, 1, 0, 3).reshape(DT, 128, FT // 2, 256))

    def btile(v, nt):
        return np.ascontiguousarray(v.reshape(nt, 128).T, f)

    z01 = np.asarray(mask[0, 0], f)  # [T,T] rows=queries, cols=keys

    shared = {
        "wq": wq_f, "wk": wk_f, "wv": wv_f, "pw": pw_f,
        "w1": w1_f, "w2": w2_f,
        "bq": btile(bq_f, DT), "bk": btile(bk_f, DT),
        "bv": np.ascontiguousarray(bv_f.reshape(1, D).astype(bf)),
        "pb": btile(proj_b.astype(f), DT),
        "b1": btile(b1_f, FT), "b2": btile(b2.astype(f), DT),
    }
    in_maps = []
    for core in range(B * ncg):
        b, j = core // ncg, core % ncg
        perm = core_perm(c, j)
        ptok = np.concatenate([np.arange(t * 128, (t + 1) * 128)
                               for t in perm])
        qtok = np.concatenate([np.arange((4 * i + j) * 128,
                                         (4 * i + j + 1) * 128)
                               for i in range(TQ // 128)])
        m = dict(shared)
        m["xT"] = np.ascontiguousarray(x[b][ptok].T, f)            # [D, TB]
        m["xqT"] = np.ascontiguousarray(x[b][qtok].T, f)           # [D, TQ]
        mm = np.empty((128, c.NTT, 128), np.float32)
        for kt in range(c.NTT):
            a = kt // 4
            gk, gq = perm[kt], 4 * a + j
            mm[:, kt, :] = z01[gq * 128:(gq + 1) * 128,
                               gk * 128:(gk + 1) * 128].T
        m["maskM"] = mm.astype(bf)
        in_maps.append(m)
    return in_maps


def assemble_out(c: CFG, results, B):
    ncg = c.TB // c.TQ
    out = np.empty((B, c.TB, c.D), np.float32)
    for core, res in enumerate(results):
        b, j = core // ncg, core % ncg
        o = res["outT"].T                                   # [TQ, D]
        for i in range(c.TQ // 128):
            t = 4 * i + j
            out[b, t * 128:(t + 1) * 128, :] = o[i * 128:(i + 1) * 128, :]
    return out


def kernel(x, mask, ln1_g, ln1_b, qkv_w, qkv_b, proj_w, proj_b,
           ln2_g, ln2_b, w1, b1, w2, b2):
    x = np.asarray(x, np.float32)
    c = CFG(D=x.shape[2], TB=x.shape[1], TQ=x.shape[1] // 4,
            NH=16, HD=64, HFF=4 * x.shape[2])
    nc = _get_nc(c)
    in_maps = make_in_maps(c, x, np.asarray(mask), *[np.asarray(a, np.float32)
                           for a in (ln1_g, ln1_b, qkv_w, qkv_b, proj_w, proj_b,
                                     ln2_g, ln2_b, w1, b1, w2, b2)])
    res = run_bass_kernel_spmd(nc, in_maps, core_ids=list(range(len(in_maps))))
    return assemble_out(c, res.results, x.shape[0])


if __name__ == "__main__":
    c = CFG()
    nc = build_nc(c)
    print("built ok")
